# revision 1
# baseline (speedup 1.0000x reference)
"""BertAttention (QKV proj + MHA + output proj + residual + LayerNorm) on 8 TRN2 NeuronCores.

Sharding: batch (4-way) x query-sequence-half (2-way) => 8 shards, no collectives.
Core c handles batch b=c//2, query half c%2. Each core computes K/V for its full
batch sequence (all heads) and Q/attention/output-proj/LayerNorm for its 1024
query rows. K/V projection work is duplicated across the 2 cores sharing a batch;
in exchange there is zero cross-core communication.

The host permutes each core's X rows so its query half comes first — attention is
permutation-invariant over keys as long as (K, V, mask) share the permutation, so
the program is identical across cores (pure SPMD) with no per-core indices.

Layouts (SBUF partition dim first):
  Xt, Kt:  [128, H/128, S]   transposed activations (feature on partitions), bf16
  Qt:      [128, H/128, SH]  transposed, bf16
  V:       [128, S/128, NH*65] natural ([tok, head-dim]) with a ones column per
           head at slot 64 — the PV matmul then yields sum(exp) as row 64 for free
  scores:  St[ktok, qtok] in PSUM; softmax sum over ktok (the partition dim) comes
           from the ones-column trick; max-subtraction safely skipped (|s| <~ 1)
  ctx:     [128, NH/2, SH]   transposed (head dim on partitions), bf16
  out:     natural [qtok, H] — residual add + LayerNorm along the free dim.
"""

from contextlib import ExitStack

import numpy as np

import bass_rust
import concourse.bass as bass
import concourse.mybir as mybir
from concourse.tile import TileContext
from concourse.bass_utils import run_bass_kernel_spmd
from concourse.masks import make_identity

FP = mybir.dt.float32
BF = mybir.dt.bfloat16
AF = mybir.ActivationFunctionType
OP = mybir.AluOpType

N_CORES = 8
EPS = 1e-12

# The walrus build in this toolchain rejects instructions that carry more than
# one sync-wait command ("Too many sync wait commands", CoreV2/V3 setupSyncWait),
# while Tile freely attaches several semaphore waits to one instruction (and the
# TileContext exit drain aggregates one wait per logical processor). Hoist the
# excess waits onto standalone InstEventSemaphore carriers on the same engine,
# placed immediately before the instruction — engine streams are serial, so the
# gating semantics are identical.
_MAX_WAITS_PER_INST = 1


def _split_sync_waits(nc, cap=_MAX_WAITS_PER_INST):
    n_split = 0
    for fn in nc.m.functions:
        for bb in fn.blocks:
            insts = list(bb.instructions)
            out = []
            changed = False
            for ins in insts:
                si = ins.sync_info
                waits = list(si.on_wait) if (si is not None and si.on_wait) else []
                if len(waits) > cap:
                    head, tail = waits[: len(waits) - cap], waits[len(waits) - cap :]
                    for j, w in enumerate(head):
                        ev = mybir.InstEventSemaphore(
                            name=f"{ins.name}-sw{j}",
                            engine=ins.engine,
                            ins=[],
                            outs=[],
                            sync_info=bass_rust.SyncInfo(on_wait=[w], on_update=[]),
                        )
                        out.append(ev)
                        n_split += 1
                    si.on_wait = tail
                    changed = True
                out.append(ins)
            if changed:
                bb.instructions[:] = out
    return n_split


def _dram_row_bcast(handle, p, n):
    """AP reading DRAM vector [n] broadcast across p partitions."""
    return bass.AP(tensor=handle, offset=0, ap=[[0, p], [1, n]])


def _build(s, h, nh, sh, flags, split=True, stop_after=None):
    """Build the per-core Bass program. flags: which bias/affine inputs matter."""
    hd = h // nh
    assert hd == 64, "head packing assumes head_dim 64 (2 heads per 128 partitions)"
    kt_n = h // 128  # contraction tiles over hidden dim
    tt_n = s // 128  # key-token tiles
    qt_n = sh // 128  # query-token tiles
    qc = min(512, sh)  # matmul moving-dim chunk over query tokens
    scale = 1.0 / float(np.sqrt(hd))

    nc = bass.Bass(target_bir_lowering=False)
    x = nc.dram_tensor("x", [s, h], FP, kind="ExternalInput")
    mask = nc.dram_tensor("mask", [s], FP, kind="ExternalInput")
    w_dram = {
        n: nc.dram_tensor(n, [h, h], FP, kind="ExternalInput")
        for n in ("wq", "wk", "wv", "wo")
    }
    vec_dram = {
        n: nc.dram_tensor(n, [h], FP, kind="ExternalInput")
        for n in ("bq", "bk", "bv", "bo", "ln_gamma", "ln_beta")
        if flags[n]
    }
    out = nc.dram_tensor("out", [sh, h], FP, kind="ExternalOutput")

    with TileContext(nc) as tc, ExitStack() as st_all:
        persist = st_all.enter_context(tc.tile_pool(name="persist", bufs=1))
        dram = st_all.enter_context(tc.tile_pool(name="dram", bufs=1, space="DRAM"))
        st_mid = st_all.enter_context(ExitStack())
        # attention-phase SBUF pools allocated low in the stack so they do not
        # overlap the released weight/X zones (which would serialize phases)
        psb = st_mid.enter_context(tc.tile_pool(name="psb", bufs=2))
        rpool = st_mid.enter_context(tc.tile_pool(name="rpool", bufs=2))

        qt = persist.tile([128, kt_n, sh], BF)
        kt = persist.tile([128, kt_n, s], BF)
        vsb = persist.tile([128, tt_n, nh * 65], BF)
        ctx_t = persist.tile([128, nh // 2, sh], BF)
        wo_bf = persist.tile([128, kt_n, h], BF)
        mask_sb = persist.tile([128, tt_n], FP)
        eps_sb = persist.tile([128, 1], FP)

        nc.vector.memset(eps_sb, EPS)
        nc.sync.dma_start(out=mask_sb, in_=mask[:].rearrange("(t p) -> p t", p=128))

        # bias columns for Qt/Kt evictions (partition = output feature in tile)
        bias_cols = {}
        for name in ("bq", "bk"):
            if flags[name]:
                col = persist.tile([128, kt_n], FP, name=f"{name}_col")
                nc.sync.dma_start(
                    out=col, in_=vec_dram[name][:].rearrange("(t p) -> p t", p=128)
                )
                bias_cols[name] = col
        # rows broadcast across partitions for V/out bias and LN affine
        bcast = {}
        for name in ("bv", "bo", "ln_gamma", "ln_beta"):
            if flags[name]:
                t = persist.tile([128, h], FP, name=f"{name}_bc")
                nc.sync.dma_start(out=t, in_=_dram_row_bcast(vec_dram[name], 128, h))
                bcast[name] = t

        # ones columns in V (slot 64 of each 65-wide head block)
        for m in range(tt_n):
            v_view = vsb[:, m, :].rearrange("p (a e) -> p a e", e=65)
            nc.vector.memset(v_view[:, :, 64:65], 1.0)

        ident = persist.tile([128, 128], BF)
        make_identity(nc, ident)

        with ExitStack() as st_proj:
            xtpool = st_proj.enter_context(tc.tile_pool(name="xtpool", bufs=1))
            wbuf = st_proj.enter_context(tc.tile_pool(name="wbuf", bufs=2))
            st_pp = st_proj.enter_context(ExitStack())
            stage = st_pp.enter_context(tc.tile_pool(name="stage", bufs=2))
            projps = st_pp.enter_context(
                tc.tile_pool(name="projps", bufs=2, space="PSUM")
            )
            tps = st_pp.enter_context(tc.tile_pool(name="tps", bufs=4, space="PSUM"))

            xt = xtpool.tile([128, kt_n, s], BF)

            hc = min(512, h)  # staging chunk (SBUF pressure)

            def load_weight(dname, w_bf=None):
                if w_bf is None:
                    w_bf = wbuf.tile([128, kt_n, h], BF, name=f"{dname}_bf", tag="w")
                for k in range(kt_n):
                    for c0 in range(0, h, hc):
                        stg = stage.tile([128, hc], FP, name="wstg", tag="stg")
                        nc.sync.dma_start(
                            out=stg, in_=w_dram[dname][k * 128 : (k + 1) * 128, c0 : c0 + hc]
                        )
                        nc.vector.tensor_copy(out=w_bf[:, k, c0 : c0 + hc], in_=stg)
                return w_bf

            # X: load f32, cast bf16, transpose on the (otherwise idle) TensorE —
            # this also pre-warms the PE clock gate before the projections.
            for t in range(tt_n):
                xbt = stage.tile([128, h], BF, name="xbt", tag="xbt")
                for c0 in range(0, h, hc):
                    stg = stage.tile([128, hc], FP, name="xstg", tag="stg")
                    nc.sync.dma_start(out=stg, in_=x[t * 128 : (t + 1) * 128, c0 : c0 + hc])
                    nc.vector.tensor_copy(out=xbt[:, c0 : c0 + hc], in_=stg)
                for k in range(kt_n):
                    tp = tps.tile([128, 128], BF, name="tp")
                    nc.tensor.transpose(tp, xbt[:, k * 128 : (k + 1) * 128], ident)
                    nc.scalar.activation(
                        out=xt[:, k, t * 128 : (t + 1) * 128], in_=tp, func=AF.Copy
                    )

            def kq_group(w_bf, dst, bias_col, m, n0, pool):
                """One K/Q projection PSUM group: 8 accumulating matmuls + evict."""
                n1 = min(n0 + 512, dst.shape[2])
                ps = pool.tile([128, 512], FP, name="projp", tag="projp")
                for k in range(kt_n):
                    nc.tensor.matmul(
                        ps[:, : n1 - n0],
                        w_bf[:, k, m * 128 : (m + 1) * 128],
                        xt[:, k, n0:n1],
                        start=(k == 0),
                        stop=(k == kt_n - 1),
                    )
                if bias_col is not None:
                    nc.vector.tensor_scalar_add(
                        out=dst[:, m, n0:n1],
                        in0=ps[:, : n1 - n0],
                        scalar1=bias_col[:, m : m + 1],
                    )
                else:
                    nc.vector.tensor_copy(out=dst[:, m, n0:n1], in_=ps[:, : n1 - n0])

            def v_group(wv_bf, m, n0):
                ps = projps.tile([128, 512], FP, name="projp", tag="projp")
                for k in range(kt_n):
                    nc.tensor.matmul(
                        ps,
                        xt[:, k, m * 128 : (m + 1) * 128],
                        wv_bf[:, k, n0 : n0 + 512],
                        start=(k == 0),
                        stop=(k == kt_n - 1),
                    )
                dst = vsb[:, m, :].rearrange("p (a e) -> p a e", e=65)[
                    :, n0 // 64 : n0 // 64 + 8, 0:64
                ]
                src = ps.rearrange("p (a e) -> p a e", e=64)
                if "bv" in bcast:
                    nc.vector.tensor_add(
                        out=dst,
                        in0=src,
                        in1=bcast["bv"][:, n0 : n0 + 512].rearrange(
                            "p (a e) -> p a e", e=64
                        ),
                    )
                else:
                    nc.vector.tensor_copy(out=dst, in_=src)

            # upfront: V (all chunks, chunk-major so early heads unblock first),
            # then K/Q for the first few feature tiles; the rest of K/Q is
            # interleaved into the attention loop as PE gap-filler.
            wv_bf = load_weight("wv")
            wk_bf = load_weight("wk")
            for n0 in range(0, h, 512):
                for m in range(tt_n):
                    v_group(wv_bf, m, n0)
            wq_bf = load_weight("wq")
            load_weight("wo", w_bf=wo_bf)

            n_up = 2 if kt_n <= 4 else 4
            for m in range(n_up):
                for n0 in range(0, s, 512):
                    kq_group(wk_bf, kt, bias_cols.get("bk"), m, n0, projps)
                for n0 in range(0, sh, 512):
                    kq_group(wq_bf, qt, bias_cols.get("bq"), m, n0, projps)

            fill_tasks = []
            for m in range(n_up, kt_n):
                for n0 in range(0, s, 512):
                    fill_tasks.append(("k", m, n0))
                for n0 in range(0, sh, 512):
                    fill_tasks.append(("q", m, n0))
            # pacing: tile m's groups must land before head 2m-1 starts
            pace = 10**9
            gidx = 0
            for m in range(n_up, kt_n):
                gidx += s // 512 + max(1, sh // 512)
                deadline = tt_n * max(1, 2 * m - 1)
                pace = min(pace, max(1, deadline // gidx))

            st_pp.close()  # release stage (SBUF) + projps/tps (PSUM)

            # ---- attention, with projection fill interleaved ----
            with (
                tc.tile_pool(name="stps", bufs=2, space="PSUM") as stps,
                tc.tile_pool(name="pvps", bufs=1, space="PSUM") as pvps,
                tc.tile_pool(name="fillps", bufs=2, space="PSUM") as fillps,
                tc.tile_pool(name="ctxu", bufs=2) as ctxu_pool,
            ):
                it = 0
                for hh in range(nh if stop_after != "proj" else 0):
                    mt, po = hh // 2, 64 * (hh % 2)
                    pv = pvps.tile([65, sh], FP, name="pvp")
                    for m in range(tt_n):
                        stt = stps.tile([128, sh], FP, name="stp")
                        for n0 in range(0, sh, qc):
                            nc.tensor.matmul(
                                stt[:, n0 : n0 + qc],
                                kt[po : po + 64, mt, m * 128 : (m + 1) * 128],
                                qt[po : po + 64, mt, n0 : n0 + qc],
                                start=True,
                                stop=True,
                            )
                        p = psb.tile([128, sh], BF, name="pexp")
                        nc.scalar.activation(
                            p, stt, AF.Exp, bias=mask_sb[:, m : m + 1], scale=scale
                        )
                        for n0 in range(0, sh, qc):
                            nc.tensor.matmul(
                                pv[:, n0 : n0 + qc],
                                vsb[:, m, hh * 65 : (hh + 1) * 65],
                                p[:, n0 : n0 + qc],
                                start=(m == 0),
                                stop=(m == tt_n - 1),
                            )
                        it += 1
                        if fill_tasks and it % pace == 0:
                            kind, fm, fn0 = fill_tasks.pop(0)
                            if kind == "k":
                                kq_group(wk_bf, kt, bias_cols.get("bk"), fm, fn0, fillps)
                            else:
                                kq_group(wq_bf, qt, bias_cols.get("bq"), fm, fn0, fillps)
                    # quick-free eviction: copy + reciprocal release the PV bank;
                    # the broadcast/normalize chain completes out of line.
                    ctx_u = ctxu_pool.tile([64, sh], BF, name="ctxu")
                    nc.vector.tensor_copy(out=ctx_u, in_=pv[0:64, :])
                    r = rpool.tile([1, sh], FP, name="recip")
                    nc.vector.reciprocal(r, pv[64:65, :])
                    # broadcast r across 64 partitions via a DRAM roundtrip (DMA
                    # partition-broadcast needs a DRAM source on this toolchain)
                    r_dram = dram.tile([sh], FP, name="rdram", tag="rdram", bufs=2)
                    nc.sync.dma_start(out=r_dram, in_=r)
                    rbc = rpool.tile([64, sh], FP, name="recipbc", bufs=2)
                    nc.sync.dma_start(
                        out=rbc,
                        in_=bass.AP(
                            tensor=r_dram.tensor,
                            offset=r_dram.offset,
                            ap=[[0, 64], [1, sh]],
                        ),
                    )
                    nc.vector.tensor_mul(
                        out=ctx_t[po : po + 64, mt, :], in0=ctx_u, in1=rbc
                    )
                for kind, fm, fn0 in fill_tasks:  # leftovers (debug paths)
                    if kind == "k":
                        kq_group(wk_bf, kt, bias_cols.get("bk"), fm, fn0, fillps)
                    else:
                        kq_group(wq_bf, qt, bias_cols.get("bq"), fm, fn0, fillps)

        st_mid.close()  # release psb/rpool before output phase

        # ---- output projection + residual + LayerNorm (natural layout) ----
        with (
            tc.tile_pool(name="ops", bufs=4, space="PSUM") as ops,
            tc.tile_pool(name="osb", bufs=2) as osb,
            tc.tile_pool(name="lnp", bufs=2) as lnp,
        ):
            for m in range(qt_n if stop_after in (None, 'oproj') else 0):
                pss = []
                for n0 in range(0, h, 512):
                    ps = ops.tile([128, 512], FP, name="op")
                    # ctx_t tile mt holds heads 2mt / 2mt+1 on partitions
                    # 0-63 / 64-127, exactly matching Wo rows mt*128..(mt+1)*128,
                    # so one K=128 matmul contracts both heads at once.
                    for mt in range(nh // 2):
                        nc.tensor.matmul(
                            ps,
                            ctx_t[:, mt, m * 128 : (m + 1) * 128],
                            wo_bf[:, mt, n0 : n0 + 512],
                            start=(mt == 0),
                            stop=(mt == nh // 2 - 1),
                        )
                    pss.append((n0, ps))
                xres = osb.tile([128, h], FP, name="xres")
                nc.sync.dma_start(out=xres, in_=x[m * 128 : (m + 1) * 128, :])
                o = osb.tile([128, h], FP, name="osum")
                for n0, ps in pss:
                    nc.vector.tensor_add(
                        out=o[:, n0 : n0 + 512], in0=ps, in1=xres[:, n0 : n0 + 512]
                    )
                if "bo" in bcast:
                    nc.vector.tensor_add(out=o, in0=o, in1=bcast["bo"])
                if stop_after == "oproj":
                    nc.sync.dma_start(out=out[m * 128 : (m + 1) * 128, :], in_=o)
                    continue
                nsub = (h + 511) // 512
                stats = lnp.tile([128, nsub, 6], FP, name="stats")
                for i in range(nsub):
                    nc.vector.bn_stats(
                        out=stats[:, i, :], in_=o[:, i * 512 : (i + 1) * 512]
                    )
                mv = lnp.tile([128, 2], FP, name="mv")
                nc.vector.bn_aggr(out=mv, in_=stats)
                std = lnp.tile([128, 1], FP, name="std")
                nc.scalar.activation(std, mv[:, 1:2], AF.Sqrt, bias=eps_sb)
                inv = lnp.tile([128, 1], FP, name="inv")
                nc.vector.reciprocal(inv, std)
                y = osb.tile([128, h], FP, name="yout")
                nc.vector.tensor_scalar(
                    out=y,
                    in0=o,
                    scalar1=mv[:, 0:1],
                    scalar2=inv,
                    op0=OP.subtract,
                    op1=OP.mult,
                )
                if "ln_gamma" in bcast:
                    nc.vector.tensor_mul(out=y, in0=y, in1=bcast["ln_gamma"])
                if "ln_beta" in bcast:
                    nc.vector.tensor_add(out=y, in0=y, in1=bcast["ln_beta"])
                nc.sync.dma_start(out=out[m * 128 : (m + 1) * 128, :], in_=y)
            if stop_after not in (None, 'oproj'):
                for m in range(qt_n):
                    dbg = osb.tile([128, h], FP, name="dbg", tag="xres")
                    if stop_after == "proj":
                        nc.vector.tensor_copy(out=dbg, in_=kt[:, 0, 0:h])
                    else:
                        nc.vector.tensor_copy(out=dbg[0:64, :sh], in_=ctx_t[0:64, 0, :])
                        nc.vector.tensor_copy(out=dbg[64:128, :sh], in_=ctx_t[64:128, 0, :])
                    nc.sync.dma_start(out=out[m * 128 : (m + 1) * 128, :], in_=dbg)

    if split:
        _split_sync_waits(nc)
    return nc


_NC_CACHE = {}


def _get_nc(s, h, nh, sh, flags):
    key = (s, h, nh, sh, tuple(sorted(flags.items())))
    if key not in _NC_CACHE:
        _NC_CACHE[key] = _build(s, h, nh, sh, flags)
    return _NC_CACHE[key]


def _prepare(hidden_states, attention_mask, Wq, bq, Wk, bk, Wv, bv, Wo, bo, ln_gamma, ln_beta):
    hs = np.ascontiguousarray(np.asarray(hidden_states, dtype=np.float32))
    b_, s_, h_ = hs.shape
    nh_ = h_ // 64
    sh_ = s_ // 2
    am = np.asarray(attention_mask, dtype=np.float32).reshape(b_, s_)
    flags = {
        "bq": bool(np.any(np.asarray(bq))),
        "bk": bool(np.any(np.asarray(bk))),
        "bv": bool(np.any(np.asarray(bv))),
        "bo": bool(np.any(np.asarray(bo))),
        "ln_gamma": not bool(np.all(np.asarray(ln_gamma) == 1.0)),
        "ln_beta": bool(np.any(np.asarray(ln_beta))),
    }
    nc = _get_nc(s_, h_, nh_, sh_, flags)

    f32c = lambda a: np.ascontiguousarray(np.asarray(a, dtype=np.float32))
    shared = {"wq": f32c(Wq), "wk": f32c(Wk), "wv": f32c(Wv), "wo": f32c(Wo)}
    for name, arr in (
        ("bq", bq),
        ("bk", bk),
        ("bv", bv),
        ("bo", bo),
        ("ln_gamma", ln_gamma),
        ("ln_beta", ln_beta),
    ):
        if flags[name]:
            shared[name] = f32c(arr)

    in_maps = []
    for c in range(N_CORES):
        bb, half = c // 2, c % 2
        mine = slice(half * sh_, (half + 1) * sh_)
        other = slice((1 - half) * sh_, (2 - half) * sh_)
        xp = np.ascontiguousarray(np.concatenate([hs[bb, mine], hs[bb, other]], axis=0))
        mp = np.ascontiguousarray(np.concatenate([am[bb, mine], am[bb, other]]))
        in_maps.append({"x": xp, "mask": mp, **shared})
    return nc, in_maps, (b_, s_, h_, sh_)


def _assemble(results, shape):
    b_, s_, h_, sh_ = shape
    out = np.empty((b_, s_, h_), dtype=np.float32)
    for c in range(N_CORES):
        bb, half = c // 2, c % 2
        out[bb, half * sh_ : (half + 1) * sh_] = results[c]["out"]
    return out


def kernel(**inputs) -> np.ndarray:
    nc, in_maps, shape = _prepare(**inputs)
    res = run_bass_kernel_spmd(nc, in_maps, core_ids=list(range(N_CORES)))
    return _assemble(res.results, shape)



# revision 9
# speedup vs baseline: 1.2893x; 1.2893x over previous
"""BertAttention (QKV proj + MHA + output proj + residual + LayerNorm) on 8 TRN2 NeuronCores.

Sharding: batch (4-way) x query-sequence-half (2-way) => 8 shards, no collectives.
Core c handles batch b=c//2, query half c%2. Each core computes K/V for its full
batch sequence (all heads) and Q/attention/output-proj/LayerNorm for its 1024
query rows. The host permutes each core's rows so its query half comes first —
attention is permutation-invariant over keys as long as (K, V, mask) share the
permutation, so the program is identical across cores (pure SPMD).

The kernel is scalar-engine bound: softmax Exp over [NH*S, SH] scores is ~283us
of ACT time per core and nothing else can run it.  Everything else is scheduled
to hide under it:

  - Host pre-casts/lays out all inputs (fp8e4 X^T and weights in DoubleRow pair
    layout, bf16 Wo / residual X), so there is no on-device staging phase.
  - QKV projections and PV run as fp8e4 DoubleRow matmuls (2 contraction rows
    per pass), scores stay bf16 (K=64 gets no DoubleRow win).  Weights are
    scaled x16 on host to avoid fp8 denormals; evictions scale by 1/16.
  - Only K/Q/V work for head 0 runs before the attention loop; all remaining
    projection groups are paced into the PE stream between attention tiles.
  - Softmax normalization (reciprocal + broadcast + multiply) is deferred two
    heads so the PV PSUM bank is released by a single [65,sh] copy and the PE
    never stalls on the vector engine (stalls >3.4us re-throttle the PE clock
    from 2.4 to 1.2 GHz).

Layouts (SBUF partition dim first):
  xt8:  [128, kp, 2, S]     fp8 X^T, DoubleRow pairs (feature 2*(128kp+... on
                            partitions; pair member i = feature block 2kp+i)
  kt:   [128, H/128, S]     transposed K (feature on partitions), bf16
  qt:   [128, H/128, SH]    transposed Q, bf16
  vsb:  [128, tp, 2, NH*65] fp8 V natural ([tok, head-dim]) with a ones column
                            per head at slot 64 (PV then yields sum(exp) as
                            row 64 for free); pair member i = token block 2tp+i
  ctx_t:[128, NH/2, SH]     transposed context (head dim on partitions), bf16
  out:  natural [qtok, H]   residual add + LayerNorm along the free dim.
"""

from contextlib import ExitStack

import numpy as np
import ml_dtypes

import bass_rust
import concourse.bass as bass
import concourse.mybir as mybir
from concourse.tile import TileContext
from concourse.bass_utils import run_bass_kernel_spmd

FP = mybir.dt.float32
BF = mybir.dt.bfloat16
F8 = mybir.dt.float8e4
AF = mybir.ActivationFunctionType
OP = mybir.AluOpType
DR = mybir.MatmulPerfMode.DoubleRow

E4NP = ml_dtypes.float8_e4m3
BFNP = ml_dtypes.bfloat16

N_CORES = 8
EPS = 1e-12
DEBUG_DUMP = False
WSCALE = 16.0  # host scales weights x16 before fp8 quant (dodges denormals)

# The walrus build in this toolchain rejects instructions that carry more than
# one sync-wait command ("Too many sync wait commands", CoreV2/V3 setupSyncWait),
# while Tile freely attaches several semaphore waits to one instruction (and the
# TileContext exit drain aggregates one wait per logical processor). Hoist the
# excess waits onto standalone InstEventSemaphore carriers on the same engine,
# placed immediately before the instruction — engine streams are serial, so the
# gating semantics are identical.
_MAX_WAITS_PER_INST = 1


def _split_sync_waits(nc, cap=_MAX_WAITS_PER_INST):
    n_split = 0
    for fn in nc.m.functions:
        for bb in fn.blocks:
            insts = list(bb.instructions)
            out = []
            changed = False
            for ins in insts:
                si = ins.sync_info
                waits = list(si.on_wait) if (si is not None and si.on_wait) else []
                if len(waits) > cap:
                    head, tail = waits[: len(waits) - cap], waits[len(waits) - cap :]
                    for j, w in enumerate(head):
                        ev = mybir.InstEventSemaphore(
                            name=f"{ins.name}-sw{j}",
                            engine=ins.engine,
                            ins=[],
                            outs=[],
                            sync_info=bass_rust.SyncInfo(on_wait=[w], on_update=[]),
                        )
                        out.append(ev)
                        n_split += 1
                    si.on_wait = tail
                    changed = True
                out.append(ins)
            if changed:
                bb.instructions[:] = out
    return n_split


def _dram_row_bcast(handle, p, n):
    """AP reading DRAM vector [n] broadcast across p partitions."""
    return bass.AP(tensor=handle, offset=0, ap=[[0, p], [1, n]])


def _build(s, h, nh, sh, flags, split=True):
    """Build the per-core Bass program. flags: which bias/affine inputs matter."""
    hd = h // nh
    assert hd == 64, "head packing assumes head_dim 64 (2 heads per 128 partitions)"
    kt_n = h // 128  # contraction tiles over hidden dim
    kp_n = kt_n // 2  # DoubleRow pairs over hidden dim
    tt_n = s // 128  # key-token tiles
    tp_n = tt_n // 2  # key-token pair tiles
    qt_n = sh // 128  # query-token tiles
    iscale = 1.0 / WSCALE
    scale = 1.0 / float(np.sqrt(hd))

    nc = bass.Bass(target_bir_lowering=False)
    xt8d = nc.dram_tensor("xt8", [128, kp_n * 2 * s], F8, kind="ExternalInput")
    xbd = nc.dram_tensor("xb", [sh, h], BF, kind="ExternalInput")
    mask = nc.dram_tensor("mask", [s], FP, kind="ExternalInput")
    w8d = {
        n: nc.dram_tensor(n, [128, kp_n * 2 * h], F8, kind="ExternalInput")
        for n in ("wq8", "wk8", "wv8")
    }
    wod = nc.dram_tensor("wo", [128, kt_n * h], BF, kind="ExternalInput")
    vec_dram = {
        n: nc.dram_tensor(n, [h], FP, kind="ExternalInput")
        for n in ("bq", "bk", "bv", "bo", "ln_gamma", "ln_beta")
        if flags[n]
    }
    out = nc.dram_tensor("out", [sh, h], FP, kind="ExternalOutput")
    dbg = {}
    if DEBUG_DUMP:
        dbg = {
            "kt_d": nc.dram_tensor("kt_d", [128, kt_n * s], BF, kind="ExternalOutput"),
            "qt_d": nc.dram_tensor("qt_d", [128, kt_n * sh], BF, kind="ExternalOutput"),
            "vsb_d": nc.dram_tensor("vsb_d", [128, tp_n * 2 * nh * 65], F8, kind="ExternalOutput"),
            "ctx_d": nc.dram_tensor("ctx_d", [128, (nh // 2) * sh], BF, kind="ExternalOutput"),
        }

    with TileContext(nc) as tc, ExitStack() as st_all:
        persist = st_all.enter_context(tc.tile_pool(name="persist", bufs=1))
        dram = st_all.enter_context(tc.tile_pool(name="dram", bufs=1, space="DRAM"))
        st_mid = st_all.enter_context(ExitStack())
        # attention-phase SBUF pools allocated low in the stack
        psb = st_mid.enter_context(tc.tile_pool(name="psb", bufs=2))
        cupool = st_mid.enter_context(tc.tile_pool(name="cupool", bufs=3))
        rpool = st_mid.enter_context(tc.tile_pool(name="rpool", bufs=2))
        rbpool = st_mid.enter_context(tc.tile_pool(name="rbpool", bufs=2))

        xt8 = persist.tile([128, kp_n, 2, s], F8)
        kt = persist.tile([128, kt_n, s], BF)
        qt = persist.tile([128, kt_n, sh], BF)
        vsb = persist.tile([128, tp_n, 2, nh * 65], F8)
        ctx_t = persist.tile([128, nh // 2, sh], BF)
        w8 = {n: persist.tile([128, kp_n, 2, h], F8, name=n) for n in ("wq8", "wk8", "wv8")}
        wo_bf = persist.tile([128, kt_n, h], BF)
        mask_sb = persist.tile([128, tt_n], FP)
        eps_sb = persist.tile([128, 1], FP)

        nc.vector.memset(eps_sb, EPS)
        nc.sync.dma_start(out=mask_sb, in_=mask[:].rearrange("(t p) -> p t", p=128))
        # DMA priority order: K/Q weights + X first (head 0 needs them), V, Wo.
        nc.sync.dma_start(out=xt8, in_=xt8d[:, :].rearrange("p (a b c) -> p a b c", a=kp_n, b=2))
        nc.sync.dma_start(out=w8["wk8"], in_=w8d["wk8"][:, :].rearrange("p (a b c) -> p a b c", a=kp_n, b=2))
        nc.sync.dma_start(out=w8["wq8"], in_=w8d["wq8"][:, :].rearrange("p (a b c) -> p a b c", a=kp_n, b=2))
        nc.sync.dma_start(out=w8["wv8"], in_=w8d["wv8"][:, :].rearrange("p (a b c) -> p a b c", a=kp_n, b=2))
        nc.sync.dma_start(out=wo_bf, in_=wod[:, :].rearrange("p (a c) -> p a c", a=kt_n))

        # bias columns for Qt/Kt evictions (partition = output feature in tile)
        bias_cols = {}
        for name in ("bq", "bk"):
            if flags[name]:
                col = persist.tile([128, kt_n], FP, name=f"{name}_col")
                nc.sync.dma_start(
                    out=col, in_=vec_dram[name][:].rearrange("(t p) -> p t", p=128)
                )
                bias_cols[name] = col
        # rows broadcast across partitions for V/out bias and LN affine
        bcast = {}
        for name in ("bv", "bo", "ln_gamma", "ln_beta"):
            if flags[name]:
                t = persist.tile([128, h], FP, name=f"{name}_bc")
                nc.sync.dma_start(out=t, in_=_dram_row_bcast(vec_dram[name], 128, h))
                bcast[name] = t

        # ones columns in V (slot 64 of each 65-wide head block)
        for tp in range(tp_n):
            for i in range(2):
                v_view = vsb[:, tp, i, :].rearrange("p (a e) -> p a e", e=65)
                nc.vector.memset(v_view[:, :, 64:65], 1.0)

        with (
            tc.tile_pool(name="stps", bufs=2, space="PSUM") as stps,
            tc.tile_pool(name="pvps", bufs=1, space="PSUM") as pvps,
            tc.tile_pool(name="fillps", bufs=2, space="PSUM") as fillps,
        ):

            def kq_group(wname, dst, bias_col, m, n0):
                """K/Q projection group: 4 DoubleRow matmuls + scaled evict."""
                ps = fillps.tile([128, 512], FP, name="projp", tag="projp")
                for kp in range(kp_n):
                    nc.tensor.matmul(
                        ps,
                        w8[wname][:, kp, :, m * 128 : (m + 1) * 128],
                        xt8[:, kp, :, n0 : n0 + 512],
                        start=(kp == 0),
                        stop=(kp == kp_n - 1),
                        perf_mode=DR,
                    )
                if bias_col is not None:
                    nc.vector.tensor_scalar(
                        out=dst[:, m, n0 : n0 + 512],
                        in0=ps,
                        scalar1=iscale,
                        scalar2=bias_col[:, m : m + 1],
                        op0=OP.mult,
                        op1=OP.add,
                    )
                else:
                    nc.vector.tensor_scalar_mul(
                        out=dst[:, m, n0 : n0 + 512], in0=ps, scalar1=iscale
                    )

            def v_group(m, n0):
                """V projection group for token tile m, v-columns [n0, n0+512)."""
                ps = fillps.tile([128, 512], FP, name="projp", tag="projp")
                for kp in range(kp_n):
                    nc.tensor.matmul(
                        ps,
                        xt8[:, kp, :, m * 128 : (m + 1) * 128],
                        w8["wv8"][:, kp, :, n0 : n0 + 512],
                        start=(kp == 0),
                        stop=(kp == kp_n - 1),
                        perf_mode=DR,
                    )
                dst = vsb[:, m // 2, m % 2, :].rearrange("p (a e) -> p a e", e=65)[
                    :, n0 // 64 : n0 // 64 + 8, 0:64
                ]
                src = ps.rearrange("p (a e) -> p a e", e=64)
                if "bv" in bcast:
                    nc.vector.scalar_tensor_tensor(
                        out=dst,
                        in0=src,
                        scalar=iscale,
                        in1=bcast["bv"][:, n0 : n0 + 512].rearrange(
                            "p (a e) -> p a e", e=64
                        ),
                        op0=OP.mult,
                        op1=OP.add,
                    )
                else:
                    nc.vector.tensor_scalar_mul(out=dst, in0=src, scalar1=iscale)

            done = set()

            def run_task(t):
                if t in done:
                    return
                done.add(t)
                kind = t[0]
                if kind == "k":
                    kq_group("wk8", kt, bias_cols.get("bk"), t[1], t[2])
                elif kind == "q":
                    kq_group("wq8", qt, bias_cols.get("bq"), t[1], t[2])
                else:
                    v_group(t[1], t[2])

            # upfront: K/Q feature tile 0 (head 0+1 scores) only.
            for n0 in range(0, s, 512):
                run_task(("k", 0, n0))
            for n0 in range(0, sh, 512):
                run_task(("q", 0, n0))

            # fill queue: everything else, ordered so prerequisites stay ahead
            # of the heads that need them (forced emission is the safety net).
            fills = []
            for m in range(tt_n):  # V column block 0 (heads 0-7)
                fills.append(("v", m, 0))
            for m in range(1, kt_n):
                for n0 in range(0, s, 512):
                    fills.append(("k", m, n0))
                for n0 in range(0, sh, 512):
                    fills.append(("q", m, n0))
                if m <= 4 and nh > 8:  # V column block 1 (heads 8-15) early
                    for mm in range((m - 1) * 4, min(tt_n, m * 4)):
                        fills.append(("v", mm, 512))
            fills = [t for t in fills if t not in done]

            def pace_fill(k=1):
                n = 0
                while fills and n < k:
                    t = fills.pop(0)
                    if t not in done:
                        run_task(t)
                        n += 1

            # ---- attention ----
            # Deferred softmax normalization, two heads behind the PV stream:
            # head h emits its own [65,sh] PSUM-freeing copy, the reciprocal +
            # DRAM-roundtrip broadcast for head h-1, and the ctx multiply for
            # head h-2 (whose broadcast landed a full head ago — no DVE stall).
            pend = {}  # hh -> {"cu": .., "rb": ..}

            def norm_recip(hh):
                if hh not in pend:
                    return
                st = pend[hh]
                r = rpool.tile([1, sh], FP, name="recip")
                nc.vector.reciprocal(r, st["cu"][64:65, :])
                rd = dram.tile([sh], FP, name="rdram", tag="rdram", bufs=2)
                nc.sync.dma_start(out=rd, in_=r)
                rb = rbpool.tile([64, sh], FP, name="recipbc")
                nc.sync.dma_start(
                    out=rb,
                    in_=bass.AP(
                        tensor=rd.tensor, offset=rd.offset, ap=[[0, 64], [1, sh]]
                    ),
                )
                st["rb"] = rb

            def norm_mult(hh):
                if hh not in pend:
                    return
                st = pend.pop(hh)
                mt, po = hh // 2, 64 * (hh % 2)
                nc.vector.tensor_mul(
                    out=ctx_t[po : po + 64, mt, :], in0=st["cu"][0:64, :], in1=st["rb"]
                )

            for hh in range(nh):
                mt, po = hh // 2, 64 * (hh % 2)
                vn0 = (hh // 8) * 512
                hcol = hh * 65
                pv = pvps.tile([65, sh], FP, name="pvp")
                for tp in range(tp_n):
                    # forced prerequisites for this pair's PV
                    run_task(("v", 2 * tp, vn0))
                    run_task(("v", 2 * tp + 1, vn0))
                    if tp == 0:  # scores prerequisites for this head
                        for n0 in range(0, s, 512):
                            run_task(("k", mt, n0))
                        for n0 in range(0, sh, 512):
                            run_task(("q", mt, n0))
                    p2 = psb.tile([128, 2, sh], F8, name="pexp")
                    for i in range(2):
                        m = 2 * tp + i
                        stt = stps.tile([128, sh], FP, name="stp")
                        for n0 in range(0, sh, 512):
                            nc.tensor.matmul(
                                stt[:, n0 : n0 + 512],
                                kt[po : po + 64, mt, m * 128 : (m + 1) * 128],
                                qt[po : po + 64, mt, n0 : n0 + 512],
                                start=True,
                                stop=True,
                            )
                        nc.scalar.activation(
                            p2[:, i, :], stt, AF.Exp,
                            bias=mask_sb[:, m : m + 1], scale=scale,
                        )
                    for n0 in range(0, sh, 512):
                        nc.tensor.matmul(
                            pv[:, n0 : n0 + 512],
                            vsb[:, tp, :, hcol : hcol + 65],
                            p2[:, :, n0 : n0 + 512],
                            start=(tp == 0),
                            stop=(tp == tp_n - 1),
                            perf_mode=DR,
                        )
                    if hh > 0:
                        pace_fill(1 if hh < 10 else (1 if tp % 2 else 0))
                # single f32 copy releases the PV PSUM bank; normalization is
                # deferred (runs while later heads stream).
                cu = cupool.tile([65, sh], FP, name="ctxu")
                nc.vector.tensor_copy(out=cu, in_=pv)
                pend[hh] = {"cu": cu}
                norm_recip(hh - 1)
                norm_mult(hh - 2)
            norm_recip(nh - 1)
            norm_mult(nh - 2)
            norm_mult(nh - 1)
            while fills:
                pace_fill(len(fills))

        if DEBUG_DUMP:
            nc.sync.dma_start(out=dbg["kt_d"][:, :], in_=kt.rearrange("p a c -> p (a c)"))
            nc.sync.dma_start(out=dbg["qt_d"][:, :], in_=qt.rearrange("p a c -> p (a c)"))
            nc.sync.dma_start(out=dbg["vsb_d"][:, :], in_=vsb.rearrange("p a b c -> p (a b c)"))
            nc.sync.dma_start(out=dbg["ctx_d"][:, :], in_=ctx_t.rearrange("p a c -> p (a c)"))

        st_mid.close()  # release attention pools before output phase

        # ---- output projection + residual + LayerNorm (natural layout) ----
        with (
            tc.tile_pool(name="ops", bufs=4, space="PSUM") as ops,
            tc.tile_pool(name="osb", bufs=2) as osb,
            tc.tile_pool(name="lnp", bufs=2) as lnp,
        ):
            for m in range(qt_n):
                pss = []
                for n0 in range(0, h, 512):
                    ps = ops.tile([128, 512], FP, name="op")
                    # ctx_t tile mt holds heads 2mt / 2mt+1 on partitions
                    # 0-63 / 64-127, exactly matching Wo rows mt*128..(mt+1)*128,
                    # so one K=128 matmul contracts both heads at once.
                    for mt in range(nh // 2):
                        nc.tensor.matmul(
                            ps,
                            ctx_t[:, mt, m * 128 : (m + 1) * 128],
                            wo_bf[:, mt, n0 : n0 + 512],
                            start=(mt == 0),
                            stop=(mt == nh // 2 - 1),
                        )
                    pss.append((n0, ps))
                xres = osb.tile([128, h], BF, name="xres")
                nc.sync.dma_start(out=xres, in_=xbd[m * 128 : (m + 1) * 128, :])
                o = osb.tile([128, h], FP, name="osum")
                for n0, ps in pss:
                    nc.vector.tensor_add(
                        out=o[:, n0 : n0 + 512], in0=ps, in1=xres[:, n0 : n0 + 512]
                    )
                if "bo" in bcast:
                    nc.vector.tensor_add(out=o, in0=o, in1=bcast["bo"])
                nsub = (h + 511) // 512
                stats = lnp.tile([128, nsub, 6], FP, name="stats")
                for i in range(nsub):
                    nc.vector.bn_stats(
                        out=stats[:, i, :], in_=o[:, i * 512 : (i + 1) * 512]
                    )
                mv = lnp.tile([128, 2], FP, name="mv")
                nc.vector.bn_aggr(out=mv, in_=stats)
                std = lnp.tile([128, 1], FP, name="std")
                nc.scalar.activation(std, mv[:, 1:2], AF.Sqrt, bias=eps_sb)
                inv = lnp.tile([128, 1], FP, name="inv")
                nc.vector.reciprocal(inv, std)
                y = osb.tile([128, h], FP, name="yout")
                nc.vector.tensor_scalar(
                    out=y,
                    in0=o,
                    scalar1=mv[:, 0:1],
                    scalar2=inv,
                    op0=OP.subtract,
                    op1=OP.mult,
                )
                if "ln_gamma" in bcast:
                    nc.vector.tensor_mul(out=y, in0=y, in1=bcast["ln_gamma"])
                if "ln_beta" in bcast:
                    nc.vector.tensor_add(out=y, in0=y, in1=bcast["ln_beta"])
                nc.sync.dma_start(out=out[m * 128 : (m + 1) * 128, :], in_=y)

    if split:
        _split_sync_waits(nc)
    return nc


_NC_CACHE = {}


def _get_nc(s, h, nh, sh, flags):
    key = (s, h, nh, sh, tuple(sorted(flags.items())))
    if key not in _NC_CACHE:
        _NC_CACHE[key] = _build(s, h, nh, sh, flags)
    return _NC_CACHE[key]


def _pack_pairs(wt, h):
    """[h, n] f32 -> flat [128, (h/256)*2*n] fp8 in DoubleRow pair layout:
    out[p, kp, i, :] = wt[(2*kp + i)*128 + p, :]."""
    n = wt.shape[1]
    kp_n = h // 256
    a = wt.reshape(kp_n, 2, 128, n).transpose(2, 0, 1, 3).reshape(128, -1)
    return np.ascontiguousarray(a.astype(E4NP))


def _prepare(hidden_states, attention_mask, Wq, bq, Wk, bk, Wv, bv, Wo, bo, ln_gamma, ln_beta):
    hs = np.ascontiguousarray(np.asarray(hidden_states, dtype=np.float32))
    b_, s_, h_ = hs.shape
    nh_ = h_ // 64
    sh_ = s_ // 2
    am = np.asarray(attention_mask, dtype=np.float32).reshape(b_, s_)
    flags = {
        "bq": bool(np.any(np.asarray(bq))),
        "bk": bool(np.any(np.asarray(bk))),
        "bv": bool(np.any(np.asarray(bv))),
        "bo": bool(np.any(np.asarray(bo))),
        "ln_gamma": not bool(np.all(np.asarray(ln_gamma) == 1.0)),
        "ln_beta": bool(np.any(np.asarray(ln_beta))),
    }
    nc = _get_nc(s_, h_, nh_, sh_, flags)

    f32c = lambda a: np.ascontiguousarray(np.asarray(a, dtype=np.float32))
    kt_n = h_ // 128
    shared = {
        "wq8": _pack_pairs(f32c(Wq) * WSCALE, h_),
        "wk8": _pack_pairs(f32c(Wk) * WSCALE, h_),
        "wv8": _pack_pairs(f32c(Wv) * WSCALE, h_),
        "wo": np.ascontiguousarray(
            f32c(Wo).reshape(kt_n, 128, h_).transpose(1, 0, 2).reshape(128, -1).astype(BFNP)
        ),
    }
    for name, arr in (
        ("bq", bq),
        ("bk", bk),
        ("bv", bv),
        ("bo", bo),
        ("ln_gamma", ln_gamma),
        ("ln_beta", ln_beta),
    ):
        if flags[name]:
            shared[name] = f32c(arr)

    in_maps = []
    for c in range(N_CORES):
        bb, half = c // 2, c % 2
        mine = slice(half * sh_, (half + 1) * sh_)
        other = slice((1 - half) * sh_, (2 - half) * sh_)
        xp = np.concatenate([hs[bb, mine], hs[bb, other]], axis=0)  # [s, h] f32
        mp = np.ascontiguousarray(np.concatenate([am[bb, mine], am[bb, other]]))
        in_maps.append(
            {
                "xt8": _pack_pairs(np.ascontiguousarray(xp.T), h_),
                "xb": np.ascontiguousarray(xp[:sh_].astype(BFNP)),
                "mask": mp,
                **shared,
            }
        )
    return nc, in_maps, (b_, s_, h_, sh_)


def _assemble(results, shape):
    b_, s_, h_, sh_ = shape
    out = np.empty((b_, s_, h_), dtype=np.float32)
    for c in range(N_CORES):
        bb, half = c // 2, c % 2
        out[bb, half * sh_ : (half + 1) * sh_] = results[c]["out"]
    return out


def kernel(**inputs) -> np.ndarray:
    nc, in_maps, shape = _prepare(**inputs)
    res = run_bass_kernel_spmd(nc, in_maps, core_ids=list(range(N_CORES)))
    return _assemble(res.results, shape)


# revision 12
# speedup vs baseline: 1.5672x; 1.2155x over previous
"""BertAttention (QKV proj + MHA + output proj + residual + LayerNorm) on 8 TRN2 NeuronCores.

Sharding: batch (4-way) x query-sequence-half (2-way) => 8 shards, no collectives.
Core c handles batch b=c//2, query half c%2. Each core computes K/V for its full
batch sequence (all heads) and Q/attention/output-proj/LayerNorm for its 1024
query rows. The host permutes each core's rows so its query half comes first —
attention is permutation-invariant over keys as long as (K, V, mask) share the
permutation, so the program is identical across cores (pure SPMD).

The kernel is scalar-engine bound: softmax Exp over [NH*S, SH] scores is ~283us
of ACT time per core and nothing else can run it.  Everything else is scheduled
to hide under it:

  - Host pre-casts/lays out all inputs (fp8e4 X^T and weights in DoubleRow pair
    layout, bf16 Wo / residual X), so there is no on-device staging phase.
  - QKV projections and PV run as fp8e4 DoubleRow matmuls (2 contraction rows
    per pass), scores stay bf16 (K=64 gets no DoubleRow win).  Weights are
    scaled x16 on host to avoid fp8 denormals; evictions scale by 1/16.
  - Only K/Q/V work for head 0 runs before the attention loop; all remaining
    projection groups are paced into the PE stream between attention tiles.
  - Softmax normalization (reciprocal + broadcast + multiply) is deferred two
    heads so the PV PSUM bank is released by a single [65,sh] copy and the PE
    never stalls on the vector engine (stalls >3.4us re-throttle the PE clock
    from 2.4 to 1.2 GHz).

Layouts (SBUF partition dim first):
  xt8:  [128, kp, 2, S]     fp8 X^T, DoubleRow pairs (feature 2*(128kp+... on
                            partitions; pair member i = feature block 2kp+i)
  kt:   [128, H/128, S]     transposed K (feature on partitions), bf16
  qt:   [128, H/128, SH]    transposed Q, bf16
  vsb:  [128, tp, 2, NH*65] fp8 V natural ([tok, head-dim]) with a ones column
                            per head at slot 64 (PV then yields sum(exp) as
                            row 64 for free); pair member i = token block 2tp+i
  ctx_t:[128, NH/2, SH]     transposed context (head dim on partitions), bf16
  out:  natural [qtok, H]   residual add + LayerNorm along the free dim.
"""

from contextlib import ExitStack

import numpy as np
import ml_dtypes

import bass_rust
import concourse.bass as bass
import concourse.mybir as mybir
from concourse.tile import TileContext
from concourse.bass_utils import run_bass_kernel_spmd

FP = mybir.dt.float32
BF = mybir.dt.bfloat16
F8 = mybir.dt.float8e4
AF = mybir.ActivationFunctionType
OP = mybir.AluOpType
DR = mybir.MatmulPerfMode.DoubleRow

E4NP = ml_dtypes.float8_e4m3
BFNP = ml_dtypes.bfloat16

N_CORES = 8
EPS = 1e-12
DEBUG_DUMP = False
WSCALE = 16.0  # host scales weights x16 before fp8 quant (dodges denormals)
CTX_SCALE = 64.0  # ctx values (~0.03) scaled into fp8 normal range for O-proj

# The walrus build in this toolchain rejects instructions that carry more than
# one sync-wait command ("Too many sync wait commands", CoreV2/V3 setupSyncWait),
# while Tile freely attaches several semaphore waits to one instruction (and the
# TileContext exit drain aggregates one wait per logical processor). Hoist the
# excess waits onto standalone InstEventSemaphore carriers on the same engine,
# placed immediately before the instruction — engine streams are serial, so the
# gating semantics are identical.
_MAX_WAITS_PER_INST = 1


def _split_sync_waits(nc, cap=_MAX_WAITS_PER_INST):
    n_split = 0
    for fn in nc.m.functions:
        for bb in fn.blocks:
            insts = list(bb.instructions)
            out = []
            changed = False
            for ins in insts:
                si = ins.sync_info
                waits = list(si.on_wait) if (si is not None and si.on_wait) else []
                if len(waits) > cap:
                    head, tail = waits[: len(waits) - cap], waits[len(waits) - cap :]
                    for j, w in enumerate(head):
                        ev = mybir.InstEventSemaphore(
                            name=f"{ins.name}-sw{j}",
                            engine=ins.engine,
                            ins=[],
                            outs=[],
                            sync_info=bass_rust.SyncInfo(on_wait=[w], on_update=[]),
                        )
                        out.append(ev)
                        n_split += 1
                    si.on_wait = tail
                    changed = True
                out.append(ins)
            if changed:
                bb.instructions[:] = out
    return n_split


def _dram_row_bcast(handle, p, n):
    """AP reading DRAM vector [n] broadcast across p partitions."""
    return bass.AP(tensor=handle, offset=0, ap=[[0, p], [1, n]])


def _build(s, h, nh, sh, flags, split=True):
    """Build the per-core Bass program. flags: which bias/affine inputs matter."""
    hd = h // nh
    assert hd == 64, "head packing assumes head_dim 64 (2 heads per 128 partitions)"
    kt_n = h // 128  # contraction tiles over hidden dim
    kp_n = kt_n // 2  # DoubleRow pairs over hidden dim
    tt_n = s // 128  # key-token tiles
    tp_n = tt_n // 2  # key-token pair tiles
    qt_n = sh // 128  # query-token tiles
    iscale = 1.0 / WSCALE
    scale = 1.0 / float(np.sqrt(hd))

    nc = bass.Bass(target_bir_lowering=False)
    xt8d = nc.dram_tensor("xt8", [128, kp_n * 2 * s], F8, kind="ExternalInput")
    xbd = nc.dram_tensor("xb", [sh, h], BF, kind="ExternalInput")
    mask = nc.dram_tensor("mask", [s], FP, kind="ExternalInput")
    w8d = {
        n: nc.dram_tensor(n, [128, kp_n * 2 * h], F8, kind="ExternalInput")
        for n in ("wq8", "wk8", "wv8")
    }
    wod = nc.dram_tensor("wo8", [128, kp_n * 2 * h], F8, kind="ExternalInput")
    vec_dram = {
        n: nc.dram_tensor(n, [h], FP, kind="ExternalInput")
        for n in ("bq", "bk", "bv", "bo", "ln_gamma", "ln_beta")
        if flags[n]
    }
    out = nc.dram_tensor("out", [sh, h], FP, kind="ExternalOutput")
    dbg = {}
    if DEBUG_DUMP:
        dbg = {
            "kt_d": nc.dram_tensor("kt_d", [128, kt_n * s], BF, kind="ExternalOutput"),
            "qt_d": nc.dram_tensor("qt_d", [128, kt_n * sh], BF, kind="ExternalOutput"),
            "vsb_d": nc.dram_tensor("vsb_d", [128, tp_n * 2 * nh * 65], F8, kind="ExternalOutput"),
            "ctx_d": nc.dram_tensor("ctx_d", [128, (nh // 2) * sh], F8, kind="ExternalOutput"),
        }

    with TileContext(nc) as tc, ExitStack() as st_all:
        persist = st_all.enter_context(tc.tile_pool(name="persist", bufs=1))
        dram = st_all.enter_context(tc.tile_pool(name="dram", bufs=1, space="DRAM"))
        st_mid = st_all.enter_context(ExitStack())
        # attention-phase SBUF pools allocated low in the stack
        psb = st_mid.enter_context(tc.tile_pool(name="psb", bufs=2))
        cupool = st_mid.enter_context(tc.tile_pool(name="cupool", bufs=6))
        rpool = st_mid.enter_context(tc.tile_pool(name="rpool", bufs=2))
        rbpool = st_mid.enter_context(tc.tile_pool(name="rbpool", bufs=2))

        xt8 = persist.tile([128, kp_n, 2, s], F8)
        kt = persist.tile([128, kt_n, s], BF)
        qt = persist.tile([128, kt_n, sh], BF)
        vsb = persist.tile([128, tp_n, 2, nh * 65], F8)
        ctx_t = persist.tile([128, nh // 2, sh], F8)  # holds 64*ctx (fp8 range)
        # per-head sum(exp) rows for batched recip: partitions 0-3 = head%4,
        # free-dim slot = head//4 (engine ops need 32-aligned partition bases)
        dall = persist.tile([4, 4, sh], FP)
        w8 = {n: persist.tile([128, kp_n, 2, h], F8, name=n) for n in ("wq8", "wk8", "wv8")}
        wo8 = persist.tile([128, kp_n, 2, h], F8)
        mask_sb = persist.tile([128, tt_n], FP)
        eps_sb = persist.tile([128, 1], FP)

        nc.vector.memset(eps_sb, EPS)
        nc.sync.dma_start(out=mask_sb, in_=mask[:].rearrange("(t p) -> p t", p=128))
        # DMA priority order: K/Q weights + X first (head 0 needs them), V, Wo.
        nc.sync.dma_start(out=xt8, in_=xt8d[:, :].rearrange("p (a b c) -> p a b c", a=kp_n, b=2))
        nc.sync.dma_start(out=w8["wk8"], in_=w8d["wk8"][:, :].rearrange("p (a b c) -> p a b c", a=kp_n, b=2))
        nc.sync.dma_start(out=w8["wq8"], in_=w8d["wq8"][:, :].rearrange("p (a b c) -> p a b c", a=kp_n, b=2))
        nc.sync.dma_start(out=w8["wv8"], in_=w8d["wv8"][:, :].rearrange("p (a b c) -> p a b c", a=kp_n, b=2))
        nc.sync.dma_start(out=wo8, in_=wod[:, :].rearrange("p (a b c) -> p a b c", a=kp_n, b=2))

        # bias columns for Qt/Kt evictions (partition = output feature in tile)
        bias_cols = {}
        for name in ("bq", "bk"):
            if flags[name]:
                col = persist.tile([128, kt_n], FP, name=f"{name}_col")
                nc.sync.dma_start(
                    out=col, in_=vec_dram[name][:].rearrange("(t p) -> p t", p=128)
                )
                bias_cols[name] = col
        # rows broadcast across partitions for V/out bias and LN affine
        bcast = {}
        for name in ("bv", "bo", "ln_gamma", "ln_beta"):
            if flags[name]:
                t = persist.tile([128, h], FP, name=f"{name}_bc")
                nc.sync.dma_start(out=t, in_=_dram_row_bcast(vec_dram[name], 128, h))
                bcast[name] = t

        # ones columns in V (slot 64 of each 65-wide head block)
        for tp in range(tp_n):
            for i in range(2):
                v_view = vsb[:, tp, i, :].rearrange("p (a e) -> p a e", e=65)
                nc.vector.memset(v_view[:, :, 64:65], 1.0)

        with (
            tc.tile_pool(name="stps", bufs=2, space="PSUM") as stps,
            tc.tile_pool(name="pvps", bufs=1, space="PSUM") as pvps,
            tc.tile_pool(name="fillps", bufs=2, space="PSUM") as fillps,
        ):

            def kq_group(wname, dst, bias_col, m, n0):
                """K/Q projection group: 4 DoubleRow matmuls + scaled evict."""
                ps = fillps.tile([128, 512], FP, name="projp", tag="projp")
                for kp in range(kp_n):
                    nc.tensor.matmul(
                        ps,
                        w8[wname][:, kp, :, m * 128 : (m + 1) * 128],
                        xt8[:, kp, :, n0 : n0 + 512],
                        start=(kp == 0),
                        stop=(kp == kp_n - 1),
                        perf_mode=DR,
                    )
                if bias_col is not None:
                    nc.vector.tensor_scalar(
                        out=dst[:, m, n0 : n0 + 512],
                        in0=ps,
                        scalar1=iscale,
                        scalar2=bias_col[:, m : m + 1],
                        op0=OP.mult,
                        op1=OP.add,
                    )
                else:
                    nc.vector.tensor_scalar_mul(
                        out=dst[:, m, n0 : n0 + 512], in0=ps, scalar1=iscale
                    )

            def v_group(m, n0):
                """V projection group for token tile m, v-columns [n0, n0+512)."""
                ps = fillps.tile([128, 512], FP, name="projp", tag="projp")
                for kp in range(kp_n):
                    nc.tensor.matmul(
                        ps,
                        xt8[:, kp, :, m * 128 : (m + 1) * 128],
                        w8["wv8"][:, kp, :, n0 : n0 + 512],
                        start=(kp == 0),
                        stop=(kp == kp_n - 1),
                        perf_mode=DR,
                    )
                dst = vsb[:, m // 2, m % 2, :].rearrange("p (a e) -> p a e", e=65)[
                    :, n0 // 64 : n0 // 64 + 8, 0:64
                ]
                src = ps.rearrange("p (a e) -> p a e", e=64)
                if "bv" in bcast:
                    nc.vector.scalar_tensor_tensor(
                        out=dst,
                        in0=src,
                        scalar=iscale,
                        in1=bcast["bv"][:, n0 : n0 + 512].rearrange(
                            "p (a e) -> p a e", e=64
                        ),
                        op0=OP.mult,
                        op1=OP.add,
                    )
                else:
                    nc.vector.tensor_scalar_mul(out=dst, in0=src, scalar1=iscale)

            done = set()

            def run_task(t):
                if t in done:
                    return
                done.add(t)
                kind = t[0]
                if kind == "k":
                    kq_group("wk8", kt, bias_cols.get("bk"), t[1], t[2])
                elif kind == "q":
                    kq_group("wq8", qt, bias_cols.get("bq"), t[1], t[2])
                else:
                    v_group(t[1], t[2])

            # upfront: K/Q feature tile 0 (head 0+1 scores) only.
            for n0 in range(0, s, 512):
                run_task(("k", 0, n0))
            for n0 in range(0, sh, 512):
                run_task(("q", 0, n0))

            # fill queue: everything else, ordered so prerequisites stay ahead
            # of the heads that need them (forced emission is the safety net).
            fills = []
            for m in range(tt_n):  # V column block 0 (heads 0-7)
                fills.append(("v", m, 0))
            for m in range(1, kt_n):
                for n0 in range(0, s, 512):
                    fills.append(("k", m, n0))
                for n0 in range(0, sh, 512):
                    fills.append(("q", m, n0))
                if m <= 4 and nh > 8:  # V column block 1 (heads 8-15) early
                    for mm in range((m - 1) * 4, min(tt_n, m * 4)):
                        fills.append(("v", mm, 512))
            fills = [t for t in fills if t not in done]

            def pace_fill(k=1):
                n = 0
                while fills and n < k:
                    t = fills.pop(0)
                    if t not in done:
                        run_task(t)
                        n += 1

            # ---- attention ----
            # Deferred softmax normalization: each head frees its PV PSUM bank
            # with one [65,sh] copy and stashes the sum(exp) row; after every
            # 4th head ONE batched reciprocal (4 partitions in parallel) + a
            # DRAM-roundtrip broadcast + 4 ctx multiplies run, interleaved into
            # the NEXT head's pair loop so the DVE queue never blocks PV.
            from collections import deque

            pend = {}  # hh -> cu tile
            norm_tasks = deque()

            def group_norm(g):
                """Queue normalize work for heads 4g..4g+3 (denoms in dall)."""
                def t_recip():
                    r = rpool.tile([4, sh], FP, name="recip")
                    nc.vector.reciprocal(r, dall[0:4, g, :])
                    rd = dram.tile([4, sh], FP, name="rdram", tag="rdram", bufs=2)
                    nc.sync.dma_start(out=rd, in_=r)
                    pend[("rd", g)] = rd
                norm_tasks.append(t_recip)

                def t_mult(hh):
                    rd = pend[("rd", g)]
                    rb = rbpool.tile([64, sh], FP, name="recipbc")
                    nc.sync.dma_start(
                        out=rb,
                        in_=bass.AP(
                            tensor=rd.tensor,
                            offset=rd.offset + (hh - 4 * g) * sh,
                            ap=[[0, 64], [1, sh]],
                        ),
                    )
                    mt, po = hh // 2, 64 * (hh % 2)
                    nc.vector.scalar_tensor_tensor(
                        out=ctx_t[po : po + 64, mt, :],
                        in0=pend.pop(hh)[0:64, :],
                        scalar=float(CTX_SCALE),
                        in1=rb,
                        op0=OP.mult,
                        op1=OP.mult,
                    )
                for hh in range(4 * g, 4 * g + 4):
                    norm_tasks.append(lambda hh=hh: t_mult(hh))

            for hh in range(nh):
                mt, po = hh // 2, 64 * (hh % 2)
                vn0 = (hh // 8) * 512
                hcol = hh * 65
                pv = pvps.tile([65, sh], FP, name="pvp")
                for tp in range(tp_n):
                    # forced prerequisites for this pair's PV
                    run_task(("v", 2 * tp, vn0))
                    run_task(("v", 2 * tp + 1, vn0))
                    if tp == 0:  # scores prerequisites for this head
                        for n0 in range(0, s, 512):
                            run_task(("k", mt, n0))
                        for n0 in range(0, sh, 512):
                            run_task(("q", mt, n0))
                    p2 = psb.tile([128, 2, sh], F8, name="pexp")
                    for i in range(2):
                        m = 2 * tp + i
                        stt = stps.tile([128, sh], FP, name="stp")
                        for n0 in range(0, sh, 512):
                            nc.tensor.matmul(
                                stt[:, n0 : n0 + 512],
                                kt[po : po + 64, mt, m * 128 : (m + 1) * 128],
                                qt[po : po + 64, mt, n0 : n0 + 512],
                                start=True,
                                stop=True,
                            )
                        nc.scalar.activation(
                            p2[:, i, :], stt, AF.Exp,
                            bias=mask_sb[:, m : m + 1], scale=scale,
                        )
                    for n0 in range(0, sh, 512):
                        nc.tensor.matmul(
                            pv[:, n0 : n0 + 512],
                            vsb[:, tp, :, hcol : hcol + 65],
                            p2[:, :, n0 : n0 + 512],
                            start=(tp == 0),
                            stop=(tp == tp_n - 1),
                            perf_mode=DR,
                        )
                    if hh > 0:
                        pace_fill(2 if hh < 6 else 1)
                    if norm_tasks:
                        norm_tasks.popleft()()
                # single f32 copy releases the PV PSUM bank; normalization is
                # deferred (runs while later heads stream).
                cu = cupool.tile([65, sh], FP, name="ctxu")
                nc.vector.tensor_copy(out=cu, in_=pv)
                pend[hh] = cu
                nc.sync.dma_start(out=dall[hh % 4 : hh % 4 + 1, hh // 4, :], in_=cu[64:65, :])
                if hh % 4 == 3:
                    group_norm(hh // 4)
            while norm_tasks:
                norm_tasks.popleft()()
            while fills:
                pace_fill(len(fills))

        if DEBUG_DUMP:
            nc.sync.dma_start(out=dbg["kt_d"][:, :], in_=kt.rearrange("p a c -> p (a c)"))
            nc.sync.dma_start(out=dbg["qt_d"][:, :], in_=qt.rearrange("p a c -> p (a c)"))
            nc.sync.dma_start(out=dbg["vsb_d"][:, :], in_=vsb.rearrange("p a b c -> p (a b c)"))
            nc.sync.dma_start(out=dbg["ctx_d"][:, :], in_=ctx_t.rearrange("p a c -> p (a c)"))

        st_mid.close()  # release attention pools before output phase

        # ---- output projection + residual + LayerNorm (natural layout) ----
        with (
            tc.tile_pool(name="ops", bufs=4, space="PSUM") as ops,
            tc.tile_pool(name="osb", bufs=2) as osb,
            tc.tile_pool(name="lnp", bufs=2) as lnp,
        ):
            for m in range(qt_n):
                pss = []
                for n0 in range(0, h, 512):
                    ps = ops.tile([128, 512], FP, name="op")
                    # ctx_t tile mt holds heads 2mt / 2mt+1 on partitions
                    # 0-63 / 64-127, exactly matching Wo rows mt*128..(mt+1)*128;
                    # DoubleRow pairs contract two mt tiles per pass.
                    for kp in range(kp_n):
                        nc.tensor.matmul(
                            ps,
                            ctx_t[:, 2 * kp : 2 * kp + 2, m * 128 : (m + 1) * 128],
                            wo8[:, kp, :, n0 : n0 + 512],
                            start=(kp == 0),
                            stop=(kp == kp_n - 1),
                            perf_mode=DR,
                        )
                    pss.append((n0, ps))
                xres = osb.tile([128, h], BF, name="xres")
                nc.sync.dma_start(out=xres, in_=xbd[m * 128 : (m + 1) * 128, :])
                o = osb.tile([128, h], FP, name="osum")
                for n0, ps in pss:
                    nc.vector.scalar_tensor_tensor(
                        out=o[:, n0 : n0 + 512],
                        in0=ps,
                        scalar=1.0 / (WSCALE * CTX_SCALE),
                        in1=xres[:, n0 : n0 + 512],
                        op0=OP.mult,
                        op1=OP.add,
                    )
                if "bo" in bcast:
                    nc.vector.tensor_add(out=o, in0=o, in1=bcast["bo"])
                nsub = (h + 511) // 512
                stats = lnp.tile([128, nsub, 6], FP, name="stats")
                for i in range(nsub):
                    nc.vector.bn_stats(
                        out=stats[:, i, :], in_=o[:, i * 512 : (i + 1) * 512]
                    )
                mv = lnp.tile([128, 2], FP, name="mv")
                nc.vector.bn_aggr(out=mv, in_=stats)
                std = lnp.tile([128, 1], FP, name="std")
                nc.scalar.activation(std, mv[:, 1:2], AF.Sqrt, bias=eps_sb)
                inv = lnp.tile([128, 1], FP, name="inv")
                nc.vector.reciprocal(inv, std)
                y = osb.tile([128, h], FP, name="yout")
                nc.vector.tensor_scalar(
                    out=y,
                    in0=o,
                    scalar1=mv[:, 0:1],
                    scalar2=inv,
                    op0=OP.subtract,
                    op1=OP.mult,
                )
                if "ln_gamma" in bcast:
                    nc.vector.tensor_mul(out=y, in0=y, in1=bcast["ln_gamma"])
                if "ln_beta" in bcast:
                    nc.vector.tensor_add(out=y, in0=y, in1=bcast["ln_beta"])
                nc.sync.dma_start(out=out[m * 128 : (m + 1) * 128, :], in_=y)

    if split:
        _split_sync_waits(nc)
    return nc


_NC_CACHE = {}


def _get_nc(s, h, nh, sh, flags):
    key = (s, h, nh, sh, tuple(sorted(flags.items())))
    if key not in _NC_CACHE:
        _NC_CACHE[key] = _build(s, h, nh, sh, flags)
    return _NC_CACHE[key]


def _pack_pairs(wt, h):
    """[h, n] f32 -> flat [128, (h/256)*2*n] fp8 in DoubleRow pair layout:
    out[p, kp, i, :] = wt[(2*kp + i)*128 + p, :]."""
    n = wt.shape[1]
    kp_n = h // 256
    a = wt.reshape(kp_n, 2, 128, n).transpose(2, 0, 1, 3).reshape(128, -1)
    return np.ascontiguousarray(a.astype(E4NP))


def _prepare(hidden_states, attention_mask, Wq, bq, Wk, bk, Wv, bv, Wo, bo, ln_gamma, ln_beta):
    hs = np.ascontiguousarray(np.asarray(hidden_states, dtype=np.float32))
    b_, s_, h_ = hs.shape
    nh_ = h_ // 64
    sh_ = s_ // 2
    am = np.asarray(attention_mask, dtype=np.float32).reshape(b_, s_)
    flags = {
        "bq": bool(np.any(np.asarray(bq))),
        "bk": bool(np.any(np.asarray(bk))),
        "bv": bool(np.any(np.asarray(bv))),
        "bo": bool(np.any(np.asarray(bo))),
        "ln_gamma": not bool(np.all(np.asarray(ln_gamma) == 1.0)),
        "ln_beta": bool(np.any(np.asarray(ln_beta))),
    }
    nc = _get_nc(s_, h_, nh_, sh_, flags)

    f32c = lambda a: np.ascontiguousarray(np.asarray(a, dtype=np.float32))
    shared = {
        "wq8": _pack_pairs(f32c(Wq) * WSCALE, h_),
        "wk8": _pack_pairs(f32c(Wk) * WSCALE, h_),
        "wv8": _pack_pairs(f32c(Wv) * WSCALE, h_),
        "wo8": _pack_pairs(f32c(Wo) * WSCALE, h_),
    }
    for name, arr in (
        ("bq", bq),
        ("bk", bk),
        ("bv", bv),
        ("bo", bo),
        ("ln_gamma", ln_gamma),
        ("ln_beta", ln_beta),
    ):
        if flags[name]:
            shared[name] = f32c(arr)

    in_maps = []
    for c in range(N_CORES):
        bb, half = c // 2, c % 2
        mine = slice(half * sh_, (half + 1) * sh_)
        other = slice((1 - half) * sh_, (2 - half) * sh_)
        xp = np.concatenate([hs[bb, mine], hs[bb, other]], axis=0)  # [s, h] f32
        mp = np.ascontiguousarray(np.concatenate([am[bb, mine], am[bb, other]]))
        in_maps.append(
            {
                "xt8": _pack_pairs(np.ascontiguousarray(xp.T), h_),
                "xb": np.ascontiguousarray(xp[:sh_].astype(BFNP)),
                "mask": mp,
                **shared,
            }
        )
    return nc, in_maps, (b_, s_, h_, sh_)


def _assemble(results, shape):
    b_, s_, h_, sh_ = shape
    out = np.empty((b_, s_, h_), dtype=np.float32)
    for c in range(N_CORES):
        bb, half = c // 2, c % 2
        out[bb, half * sh_ : (half + 1) * sh_] = results[c]["out"]
    return out


def kernel(**inputs) -> np.ndarray:
    nc, in_maps, shape = _prepare(**inputs)
    res = run_bass_kernel_spmd(nc, in_maps, core_ids=list(range(N_CORES)))
    return _assemble(res.results, shape)


# revision 13
# speedup vs baseline: 1.5737x; 1.0042x over previous
"""BertAttention (QKV proj + MHA + output proj + residual + LayerNorm) on 8 TRN2 NeuronCores.

Sharding: batch (4-way) x query-sequence-half (2-way) => 8 shards, no collectives.
Core c handles batch b=c//2, query half c%2. Each core computes K/V for its full
batch sequence (all heads) and Q/attention/output-proj/LayerNorm for its 1024
query rows. The host permutes each core's rows so its query half comes first —
attention is permutation-invariant over keys as long as (K, V, mask) share the
permutation, so the program is identical across cores (pure SPMD).

The kernel is scalar-engine bound: softmax Exp over [NH*S, SH] scores is ~283us
of ACT time per core and nothing else can run it.  Everything else is scheduled
to hide under it:

  - Host pre-casts/lays out all inputs (fp8e4 X^T and weights in DoubleRow pair
    layout, bf16 Wo / residual X), so there is no on-device staging phase.
  - QKV projections and PV run as fp8e4 DoubleRow matmuls (2 contraction rows
    per pass), scores stay bf16 (K=64 gets no DoubleRow win).  Weights are
    scaled x16 on host to avoid fp8 denormals; evictions scale by 1/16.
  - Only K/Q/V work for head 0 runs before the attention loop; all remaining
    projection groups are paced into the PE stream between attention tiles.
  - Softmax normalization (reciprocal + broadcast + multiply) is deferred two
    heads so the PV PSUM bank is released by a single [65,sh] copy and the PE
    never stalls on the vector engine (stalls >3.4us re-throttle the PE clock
    from 2.4 to 1.2 GHz).

Layouts (SBUF partition dim first):
  xt8:  [128, kp, 2, S]     fp8 X^T, DoubleRow pairs (feature 2*(128kp+... on
                            partitions; pair member i = feature block 2kp+i)
  kt:   [128, H/128, S]     transposed K (feature on partitions), bf16
  qt:   [128, H/128, SH]    transposed Q, bf16
  vsb:  [128, tp, 2, NH*65] fp8 V natural ([tok, head-dim]) with a ones column
                            per head at slot 64 (PV then yields sum(exp) as
                            row 64 for free); pair member i = token block 2tp+i
  ctx_t:[128, NH/2, SH]     transposed context (head dim on partitions), bf16
  out:  natural [qtok, H]   residual add + LayerNorm along the free dim.
"""

from contextlib import ExitStack

import numpy as np
import ml_dtypes

import bass_rust
import concourse.bass as bass
import concourse.mybir as mybir
from concourse.tile import TileContext
from concourse.bass_utils import run_bass_kernel_spmd

FP = mybir.dt.float32
BF = mybir.dt.bfloat16
F8 = mybir.dt.float8e4
AF = mybir.ActivationFunctionType
OP = mybir.AluOpType
DR = mybir.MatmulPerfMode.DoubleRow

E4NP = ml_dtypes.float8_e4m3
BFNP = ml_dtypes.bfloat16

N_CORES = 8
EPS = 1e-12
DEBUG_DUMP = False
WSCALE = 16.0  # host scales weights x16 before fp8 quant (dodges denormals)
CTX_SCALE = 64.0  # ctx values (~0.03) scaled into fp8 normal range for O-proj

# The walrus build in this toolchain rejects instructions that carry more than
# one sync-wait command ("Too many sync wait commands", CoreV2/V3 setupSyncWait),
# while Tile freely attaches several semaphore waits to one instruction (and the
# TileContext exit drain aggregates one wait per logical processor). Hoist the
# excess waits onto standalone InstEventSemaphore carriers on the same engine,
# placed immediately before the instruction — engine streams are serial, so the
# gating semantics are identical.
_MAX_WAITS_PER_INST = 1


def _split_sync_waits(nc, cap=_MAX_WAITS_PER_INST):
    n_split = 0
    for fn in nc.m.functions:
        for bb in fn.blocks:
            insts = list(bb.instructions)
            out = []
            changed = False
            for ins in insts:
                si = ins.sync_info
                waits = list(si.on_wait) if (si is not None and si.on_wait) else []
                if len(waits) > cap:
                    head, tail = waits[: len(waits) - cap], waits[len(waits) - cap :]
                    for j, w in enumerate(head):
                        ev = mybir.InstEventSemaphore(
                            name=f"{ins.name}-sw{j}",
                            engine=ins.engine,
                            ins=[],
                            outs=[],
                            sync_info=bass_rust.SyncInfo(on_wait=[w], on_update=[]),
                        )
                        out.append(ev)
                        n_split += 1
                    si.on_wait = tail
                    changed = True
                out.append(ins)
            if changed:
                bb.instructions[:] = out
    return n_split


def _dram_row_bcast(handle, p, n):
    """AP reading DRAM vector [n] broadcast across p partitions."""
    return bass.AP(tensor=handle, offset=0, ap=[[0, p], [1, n]])


def _build(s, h, nh, sh, flags, split=True):
    """Build the per-core Bass program. flags: which bias/affine inputs matter."""
    hd = h // nh
    assert hd == 64, "head packing assumes head_dim 64 (2 heads per 128 partitions)"
    kt_n = h // 128  # contraction tiles over hidden dim
    kp_n = kt_n // 2  # DoubleRow pairs over hidden dim
    tt_n = s // 128  # key-token tiles
    tp_n = tt_n // 2  # key-token pair tiles
    qt_n = sh // 128  # query-token tiles
    iscale = 1.0 / WSCALE
    scale = 1.0 / float(np.sqrt(hd))

    nc = bass.Bass(target_bir_lowering=False)
    xt8d = nc.dram_tensor("xt8", [128, kp_n * 2 * s], F8, kind="ExternalInput")
    xbd = nc.dram_tensor("xb", [sh, h], BF, kind="ExternalInput")
    mask = nc.dram_tensor("mask", [s], FP, kind="ExternalInput")
    w8d = {
        n: nc.dram_tensor(n, [128, kp_n * 2 * h], F8, kind="ExternalInput")
        for n in ("wq8", "wk8", "wv8")
    }
    wod = nc.dram_tensor("wo8", [128, kp_n * 2 * h], F8, kind="ExternalInput")
    vec_dram = {
        n: nc.dram_tensor(n, [h], FP, kind="ExternalInput")
        for n in ("bq", "bk", "bv", "bo", "ln_gamma", "ln_beta")
        if flags[n]
    }
    out = nc.dram_tensor("out", [sh, h], FP, kind="ExternalOutput")
    dbg = {}
    if DEBUG_DUMP:
        dbg = {
            "kt_d": nc.dram_tensor("kt_d", [128, kt_n * s], BF, kind="ExternalOutput"),
            "qt_d": nc.dram_tensor("qt_d", [128, kt_n * sh], BF, kind="ExternalOutput"),
            "vsb_d": nc.dram_tensor("vsb_d", [128, tp_n * 2 * nh * 65], F8, kind="ExternalOutput"),
            "ctx_d": nc.dram_tensor("ctx_d", [128, (nh // 2) * sh], F8, kind="ExternalOutput"),
        }

    with TileContext(nc) as tc, ExitStack() as st_all:
        persist = st_all.enter_context(tc.tile_pool(name="persist", bufs=1))
        dram = st_all.enter_context(tc.tile_pool(name="dram", bufs=1, space="DRAM"))
        st_mid = st_all.enter_context(ExitStack())
        # attention-phase SBUF pools allocated low in the stack
        psb = st_mid.enter_context(tc.tile_pool(name="psb", bufs=2))
        cupool = st_mid.enter_context(tc.tile_pool(name="cupool", bufs=4))
        rpool = st_mid.enter_context(tc.tile_pool(name="rpool", bufs=2))
        rbpool = st_mid.enter_context(tc.tile_pool(name="rbpool", bufs=2))

        xt8 = persist.tile([128, kp_n, 2, s], F8)
        kt = persist.tile([128, kt_n, s], BF)
        qt = persist.tile([128, kt_n, sh], BF)
        vsb = persist.tile([128, tp_n, 2, nh * 65], F8)
        ctx_t = persist.tile([128, nh // 2, sh], F8)  # holds 64*ctx (fp8 range)
        # per-head sum(exp) rows for batched recip: partitions 0-1 = head%2,
        # free-dim slot = head//2 (engine ops need 32-aligned partition bases)
        dall = persist.tile([2, nh // 2, sh], FP)
        w8 = {n: persist.tile([128, kp_n, 2, h], F8, name=n) for n in ("wq8", "wk8", "wv8")}
        wo8 = persist.tile([128, kp_n, 2, h], F8)
        mask_sb = persist.tile([128, tt_n], FP)
        eps_sb = persist.tile([128, 1], FP)

        nc.vector.memset(eps_sb, EPS)
        nc.sync.dma_start(out=mask_sb, in_=mask[:].rearrange("(t p) -> p t", p=128))
        # DMA priority order: K/Q weights + X first (head 0 needs them), V, Wo.
        for kp in range(kp_n):  # chunked: spread across DMA queues
            nc.sync.dma_start(
                out=xt8[:, kp],
                in_=xt8d[:, kp * 2 * s : (kp + 1) * 2 * s].rearrange("p (b c) -> p b c", b=2),
            )
            nc.sync.dma_start(
                out=w8["wk8"][:, kp],
                in_=w8d["wk8"][:, kp * 2 * h : (kp + 1) * 2 * h].rearrange("p (b c) -> p b c", b=2),
            )
        nc.sync.dma_start(out=w8["wq8"], in_=w8d["wq8"][:, :].rearrange("p (a b c) -> p a b c", a=kp_n, b=2))
        nc.sync.dma_start(out=w8["wv8"], in_=w8d["wv8"][:, :].rearrange("p (a b c) -> p a b c", a=kp_n, b=2))
        nc.sync.dma_start(out=wo8, in_=wod[:, :].rearrange("p (a b c) -> p a b c", a=kp_n, b=2))
        xres_all = persist.tile([128, qt_n, h], BF, name="xres_all")
        for mq in range(qt_n):
            nc.sync.dma_start(
                out=xres_all[:, mq], in_=xbd[mq * 128 : (mq + 1) * 128, :]
            )

        # bias columns for Qt/Kt evictions (partition = output feature in tile)
        bias_cols = {}
        for name in ("bq", "bk"):
            if flags[name]:
                col = persist.tile([128, kt_n], FP, name=f"{name}_col")
                nc.sync.dma_start(
                    out=col, in_=vec_dram[name][:].rearrange("(t p) -> p t", p=128)
                )
                bias_cols[name] = col
        # rows broadcast across partitions for V/out bias and LN affine
        bcast = {}
        for name in ("bv", "bo", "ln_gamma", "ln_beta"):
            if flags[name]:
                t = persist.tile([128, h], FP, name=f"{name}_bc")
                nc.sync.dma_start(out=t, in_=_dram_row_bcast(vec_dram[name], 128, h))
                bcast[name] = t

        # ones columns in V (slot 64 of each 65-wide head block)
        for tp in range(tp_n):
            for i in range(2):
                v_view = vsb[:, tp, i, :].rearrange("p (a e) -> p a e", e=65)
                nc.vector.memset(v_view[:, :, 64:65], 1.0)

        with (
            tc.tile_pool(name="stps", bufs=2, space="PSUM") as stps,
            tc.tile_pool(name="pvps", bufs=1, space="PSUM") as pvps,
            tc.tile_pool(name="fillps", bufs=2, space="PSUM") as fillps,
        ):

            def kq_group(wname, dst, bias_col, m, n0):
                """K/Q projection group: 4 DoubleRow matmuls + scaled evict."""
                ps = fillps.tile([128, 512], FP, name="projp", tag="projp")
                for kp in range(kp_n):
                    nc.tensor.matmul(
                        ps,
                        w8[wname][:, kp, :, m * 128 : (m + 1) * 128],
                        xt8[:, kp, :, n0 : n0 + 512],
                        start=(kp == 0),
                        stop=(kp == kp_n - 1),
                        perf_mode=DR,
                    )
                if bias_col is not None:
                    nc.vector.tensor_scalar(
                        out=dst[:, m, n0 : n0 + 512],
                        in0=ps,
                        scalar1=iscale,
                        scalar2=bias_col[:, m : m + 1],
                        op0=OP.mult,
                        op1=OP.add,
                    )
                else:
                    nc.vector.tensor_scalar_mul(
                        out=dst[:, m, n0 : n0 + 512], in0=ps, scalar1=iscale
                    )

            def v_group(m, n0):
                """V projection group for token tile m, v-columns [n0, n0+512)."""
                ps = fillps.tile([128, 512], FP, name="projp", tag="projp")
                for kp in range(kp_n):
                    nc.tensor.matmul(
                        ps,
                        xt8[:, kp, :, m * 128 : (m + 1) * 128],
                        w8["wv8"][:, kp, :, n0 : n0 + 512],
                        start=(kp == 0),
                        stop=(kp == kp_n - 1),
                        perf_mode=DR,
                    )
                dst = vsb[:, m // 2, m % 2, :].rearrange("p (a e) -> p a e", e=65)[
                    :, n0 // 64 : n0 // 64 + 8, 0:64
                ]
                src = ps.rearrange("p (a e) -> p a e", e=64)
                if "bv" in bcast:
                    nc.vector.scalar_tensor_tensor(
                        out=dst,
                        in0=src,
                        scalar=iscale,
                        in1=bcast["bv"][:, n0 : n0 + 512].rearrange(
                            "p (a e) -> p a e", e=64
                        ),
                        op0=OP.mult,
                        op1=OP.add,
                    )
                else:
                    nc.vector.tensor_scalar_mul(out=dst, in0=src, scalar1=iscale)

            done = set()

            def run_task(t):
                if t in done:
                    return
                done.add(t)
                kind = t[0]
                if kind == "k":
                    kq_group("wk8", kt, bias_cols.get("bk"), t[1], t[2])
                elif kind == "q":
                    kq_group("wq8", qt, bias_cols.get("bq"), t[1], t[2])
                else:
                    v_group(t[1], t[2])

            # upfront: K/Q feature tile 0 (head 0+1 scores) only.
            for n0 in range(0, s, 512):
                run_task(("k", 0, n0))
            for n0 in range(0, sh, 512):
                run_task(("q", 0, n0))

            # fill queue: everything else, ordered so prerequisites stay ahead
            # of the heads that need them (forced emission is the safety net).
            fills = []
            for m in range(tt_n):  # V column block 0 (heads 0-7)
                fills.append(("v", m, 0))
            for m in range(1, kt_n):
                for n0 in range(0, s, 512):
                    fills.append(("k", m, n0))
                for n0 in range(0, sh, 512):
                    fills.append(("q", m, n0))
                if m <= 4 and nh > 8:  # V column block 1 (heads 8-15) early
                    for mm in range((m - 1) * 4, min(tt_n, m * 4)):
                        fills.append(("v", mm, 512))
            fills = [t for t in fills if t not in done]

            def pace_fill(k=1):
                n = 0
                while fills and n < k:
                    t = fills.pop(0)
                    if t not in done:
                        run_task(t)
                        n += 1

            # ---- attention ----
            # Deferred softmax normalization: each head frees its PV PSUM bank
            # with one [65,sh] copy and stashes the sum(exp) row; after every
            # 4th head ONE batched reciprocal (4 partitions in parallel) + a
            # DRAM-roundtrip broadcast + 4 ctx multiplies run, interleaved into
            # the NEXT head's pair loop so the DVE queue never blocks PV.
            from collections import deque

            pend = {}  # hh -> cu tile
            norm_tasks = deque()

            def group_norm(g):
                """Queue normalize work for heads 2g, 2g+1 (denoms in dall)."""
                def t_recip():
                    r = rpool.tile([2, sh], FP, name="recip")
                    nc.vector.reciprocal(r, dall[0:2, g, :])
                    rd = dram.tile([2, sh], FP, name="rdram", tag="rdram", bufs=2)
                    nc.sync.dma_start(out=rd, in_=r)
                    pend[("rd", g)] = rd
                norm_tasks.append(t_recip)

                def t_mult(hh):
                    rd = pend[("rd", g)]
                    rb = rbpool.tile([64, sh], FP, name="recipbc")
                    nc.sync.dma_start(
                        out=rb,
                        in_=bass.AP(
                            tensor=rd.tensor,
                            offset=rd.offset + (hh - 2 * g) * sh,
                            ap=[[0, 64], [1, sh]],
                        ),
                    )
                    mt, po = hh // 2, 64 * (hh % 2)
                    nc.vector.scalar_tensor_tensor(
                        out=ctx_t[po : po + 64, mt, :],
                        in0=pend.pop(hh)[0:64, :],
                        scalar=float(CTX_SCALE),
                        in1=rb,
                        op0=OP.mult,
                        op1=OP.mult,
                    )
                for hh in range(2 * g, 2 * g + 2):
                    norm_tasks.append(lambda hh=hh: t_mult(hh))

            for hh in range(nh):
                mt, po = hh // 2, 64 * (hh % 2)
                vn0 = (hh // 8) * 512
                hcol = hh * 65
                pv = pvps.tile([65, sh], FP, name="pvp")
                for tp in range(tp_n):
                    # forced prerequisites for this pair's PV
                    run_task(("v", 2 * tp, vn0))
                    run_task(("v", 2 * tp + 1, vn0))
                    if tp == 0:  # scores prerequisites for this head
                        for n0 in range(0, s, 512):
                            run_task(("k", mt, n0))
                        for n0 in range(0, sh, 512):
                            run_task(("q", mt, n0))
                    p2 = psb.tile([128, 2, sh], F8, name="pexp")
                    for i in range(2):
                        m = 2 * tp + i
                        stt = stps.tile([128, sh], FP, name="stp")
                        for n0 in range(0, sh, 512):
                            nc.tensor.matmul(
                                stt[:, n0 : n0 + 512],
                                kt[po : po + 64, mt, m * 128 : (m + 1) * 128],
                                qt[po : po + 64, mt, n0 : n0 + 512],
                                start=True,
                                stop=True,
                            )
                        nc.scalar.activation(
                            p2[:, i, :], stt, AF.Exp,
                            bias=mask_sb[:, m : m + 1], scale=scale,
                        )
                    for n0 in range(0, sh, 512):
                        nc.tensor.matmul(
                            pv[:, n0 : n0 + 512],
                            vsb[:, tp, :, hcol : hcol + 65],
                            p2[:, :, n0 : n0 + 512],
                            start=(tp == 0),
                            stop=(tp == tp_n - 1),
                            perf_mode=DR,
                        )
                    if hh > 0:
                        pace_fill(2 if hh < 6 else 1)
                    if norm_tasks:
                        norm_tasks.popleft()()
                # single f32 copy releases the PV PSUM bank; normalization is
                # deferred (runs while later heads stream).
                cu = cupool.tile([65, sh], FP, name="ctxu")
                nc.vector.tensor_copy(out=cu, in_=pv)
                pend[hh] = cu
                nc.sync.dma_start(out=dall[hh % 2 : hh % 2 + 1, hh // 2, :], in_=cu[64:65, :])
                if hh % 2 == 1:
                    group_norm(hh // 2)
            while norm_tasks:
                norm_tasks.popleft()()
            while fills:
                pace_fill(len(fills))
            assert not pend or all(isinstance(k, tuple) for k in pend)

        if DEBUG_DUMP:
            nc.sync.dma_start(out=dbg["kt_d"][:, :], in_=kt.rearrange("p a c -> p (a c)"))
            nc.sync.dma_start(out=dbg["qt_d"][:, :], in_=qt.rearrange("p a c -> p (a c)"))
            nc.sync.dma_start(out=dbg["vsb_d"][:, :], in_=vsb.rearrange("p a b c -> p (a b c)"))
            nc.sync.dma_start(out=dbg["ctx_d"][:, :], in_=ctx_t.rearrange("p a c -> p (a c)"))

        st_mid.close()  # release attention pools before output phase

        # ---- output projection + residual + LayerNorm (natural layout) ----
        with (
            tc.tile_pool(name="ops", bufs=4, space="PSUM") as ops,
            tc.tile_pool(name="osb", bufs=2) as osb,
            tc.tile_pool(name="lnp", bufs=2) as lnp,
        ):
            for m in range(qt_n):
                pss = []
                for n0 in range(0, h, 512):
                    ps = ops.tile([128, 512], FP, name="op")
                    # ctx_t tile mt holds heads 2mt / 2mt+1 on partitions
                    # 0-63 / 64-127, exactly matching Wo rows mt*128..(mt+1)*128;
                    # DoubleRow pairs contract two mt tiles per pass.
                    for kp in range(kp_n):
                        nc.tensor.matmul(
                            ps,
                            ctx_t[:, 2 * kp : 2 * kp + 2, m * 128 : (m + 1) * 128],
                            wo8[:, kp, :, n0 : n0 + 512],
                            start=(kp == 0),
                            stop=(kp == kp_n - 1),
                            perf_mode=DR,
                        )
                    pss.append((n0, ps))
                xres = xres_all[:, m]
                o = osb.tile([128, h], FP, name="osum")
                for n0, ps in pss:
                    nc.vector.scalar_tensor_tensor(
                        out=o[:, n0 : n0 + 512],
                        in0=ps,
                        scalar=1.0 / (WSCALE * CTX_SCALE),
                        in1=xres[:, n0 : n0 + 512],
                        op0=OP.mult,
                        op1=OP.add,
                    )
                if "bo" in bcast:
                    nc.vector.tensor_add(out=o, in0=o, in1=bcast["bo"])
                nsub = (h + 511) // 512
                stats = lnp.tile([128, nsub, 6], FP, name="stats")
                for i in range(nsub):
                    nc.vector.bn_stats(
                        out=stats[:, i, :], in_=o[:, i * 512 : (i + 1) * 512]
                    )
                mv = lnp.tile([128, 2], FP, name="mv")
                nc.vector.bn_aggr(out=mv, in_=stats)
                std = lnp.tile([128, 1], FP, name="std")
                nc.scalar.activation(std, mv[:, 1:2], AF.Sqrt, bias=eps_sb)
                inv = lnp.tile([128, 1], FP, name="inv")
                nc.vector.reciprocal(inv, std)
                y = osb.tile([128, h], FP, name="yout")
                nc.vector.tensor_scalar(
                    out=y,
                    in0=o,
                    scalar1=mv[:, 0:1],
                    scalar2=inv,
                    op0=OP.subtract,
                    op1=OP.mult,
                )
                if "ln_gamma" in bcast:
                    nc.vector.tensor_mul(out=y, in0=y, in1=bcast["ln_gamma"])
                if "ln_beta" in bcast:
                    nc.vector.tensor_add(out=y, in0=y, in1=bcast["ln_beta"])
                nc.sync.dma_start(out=out[m * 128 : (m + 1) * 128, :], in_=y)

    if split:
        _split_sync_waits(nc)
    return nc


_NC_CACHE = {}


def _get_nc(s, h, nh, sh, flags):
    key = (s, h, nh, sh, tuple(sorted(flags.items())))
    if key not in _NC_CACHE:
        _NC_CACHE[key] = _build(s, h, nh, sh, flags)
    return _NC_CACHE[key]


def _pack_pairs(wt, h):
    """[h, n] f32 -> flat [128, (h/256)*2*n] fp8 in DoubleRow pair layout:
    out[p, kp, i, :] = wt[(2*kp + i)*128 + p, :]."""
    n = wt.shape[1]
    kp_n = h // 256
    a = wt.reshape(kp_n, 2, 128, n).transpose(2, 0, 1, 3).reshape(128, -1)
    return np.ascontiguousarray(a.astype(E4NP))


def _prepare(hidden_states, attention_mask, Wq, bq, Wk, bk, Wv, bv, Wo, bo, ln_gamma, ln_beta):
    hs = np.ascontiguousarray(np.asarray(hidden_states, dtype=np.float32))
    b_, s_, h_ = hs.shape
    nh_ = h_ // 64
    sh_ = s_ // 2
    am = np.asarray(attention_mask, dtype=np.float32).reshape(b_, s_)
    flags = {
        "bq": bool(np.any(np.asarray(bq))),
        "bk": bool(np.any(np.asarray(bk))),
        "bv": bool(np.any(np.asarray(bv))),
        "bo": bool(np.any(np.asarray(bo))),
        "ln_gamma": not bool(np.all(np.asarray(ln_gamma) == 1.0)),
        "ln_beta": bool(np.any(np.asarray(ln_beta))),
    }
    nc = _get_nc(s_, h_, nh_, sh_, flags)

    f32c = lambda a: np.ascontiguousarray(np.asarray(a, dtype=np.float32))
    shared = {
        "wq8": _pack_pairs(f32c(Wq) * WSCALE, h_),
        "wk8": _pack_pairs(f32c(Wk) * WSCALE, h_),
        "wv8": _pack_pairs(f32c(Wv) * WSCALE, h_),
        "wo8": _pack_pairs(f32c(Wo) * WSCALE, h_),
    }
    for name, arr in (
        ("bq", bq),
        ("bk", bk),
        ("bv", bv),
        ("bo", bo),
        ("ln_gamma", ln_gamma),
        ("ln_beta", ln_beta),
    ):
        if flags[name]:
            shared[name] = f32c(arr)

    in_maps = []
    for c in range(N_CORES):
        bb, half = c // 2, c % 2
        mine = slice(half * sh_, (half + 1) * sh_)
        other = slice((1 - half) * sh_, (2 - half) * sh_)
        xp = np.concatenate([hs[bb, mine], hs[bb, other]], axis=0)  # [s, h] f32
        mp = np.ascontiguousarray(np.concatenate([am[bb, mine], am[bb, other]]))
        in_maps.append(
            {
                "xt8": _pack_pairs(np.ascontiguousarray(xp.T), h_),
                "xb": np.ascontiguousarray(xp[:sh_].astype(BFNP)),
                "mask": mp,
                **shared,
            }
        )
    return nc, in_maps, (b_, s_, h_, sh_)


def _assemble(results, shape):
    b_, s_, h_, sh_ = shape
    out = np.empty((b_, s_, h_), dtype=np.float32)
    for c in range(N_CORES):
        bb, half = c // 2, c % 2
        out[bb, half * sh_ : (half + 1) * sh_] = results[c]["out"]
    return out


def kernel(**inputs) -> np.ndarray:
    nc, in_maps, shape = _prepare(**inputs)
    res = run_bass_kernel_spmd(nc, in_maps, core_ids=list(range(N_CORES)))
    return _assemble(res.results, shape)


# revision 14
# speedup vs baseline: 1.5871x; 1.0085x over previous
"""BertAttention (QKV proj + MHA + output proj + residual + LayerNorm) on 8 TRN2 NeuronCores.

Sharding: batch (4-way) x query-sequence-half (2-way) => 8 shards, no collectives.
Core c handles batch b=c//2, query half c%2. Each core computes K/V for its full
batch sequence (all heads) and Q/attention/output-proj/LayerNorm for its 1024
query rows. The host permutes each core's rows so its query half comes first —
attention is permutation-invariant over keys as long as (K, V, mask) share the
permutation, so the program is identical across cores (pure SPMD).

The kernel is scalar-engine bound: softmax Exp over [NH*S, SH] scores is ~283us
of ACT time per core and nothing else can run it.  Everything else is scheduled
to hide under it:

  - Host pre-casts/lays out all inputs (fp8e4 X^T and weights in DoubleRow pair
    layout, bf16 Wo / residual X), so there is no on-device staging phase.
  - QKV projections and PV run as fp8e4 DoubleRow matmuls (2 contraction rows
    per pass), scores stay bf16 (K=64 gets no DoubleRow win).  Weights are
    scaled x16 on host to avoid fp8 denormals; evictions scale by 1/16.
  - Only K/Q/V work for head 0 runs before the attention loop; all remaining
    projection groups are paced into the PE stream between attention tiles.
  - Softmax normalization (reciprocal + broadcast + multiply) is deferred two
    heads so the PV PSUM bank is released by a single [65,sh] copy and the PE
    never stalls on the vector engine (stalls >3.4us re-throttle the PE clock
    from 2.4 to 1.2 GHz).

Layouts (SBUF partition dim first):
  xt8:  [128, kp, 2, S]     fp8 X^T, DoubleRow pairs (feature 2*(128kp+... on
                            partitions; pair member i = feature block 2kp+i)
  kt:   [128, H/128, S]     transposed K (feature on partitions), bf16
  qt:   [128, H/128, SH]    transposed Q, bf16
  vsb:  [128, tp, 2, NH*65] fp8 V natural ([tok, head-dim]) with a ones column
                            per head at slot 64 (PV then yields sum(exp) as
                            row 64 for free); pair member i = token block 2tp+i
  ctx_t:[128, NH/2, SH]     transposed context (head dim on partitions), bf16
  out:  natural [qtok, H]   residual add + LayerNorm along the free dim.
"""

from contextlib import ExitStack

import numpy as np
import ml_dtypes

import bass_rust
import concourse.bass as bass
import concourse.mybir as mybir
from concourse.tile import TileContext
from concourse.bass_utils import run_bass_kernel_spmd

FP = mybir.dt.float32
BF = mybir.dt.bfloat16
F8 = mybir.dt.float8e4
AF = mybir.ActivationFunctionType
OP = mybir.AluOpType
DR = mybir.MatmulPerfMode.DoubleRow

E4NP = ml_dtypes.float8_e4m3
BFNP = ml_dtypes.bfloat16

N_CORES = 8
EPS = 1e-12
DEBUG_DUMP = False
WSCALE = 16.0  # host scales weights x16 before fp8 quant (dodges denormals)
CTX_SCALE = 64.0  # ctx values (~0.03) scaled into fp8 normal range for O-proj

# The walrus build in this toolchain rejects instructions that carry more than
# one sync-wait command ("Too many sync wait commands", CoreV2/V3 setupSyncWait),
# while Tile freely attaches several semaphore waits to one instruction (and the
# TileContext exit drain aggregates one wait per logical processor). Hoist the
# excess waits onto standalone InstEventSemaphore carriers on the same engine,
# placed immediately before the instruction — engine streams are serial, so the
# gating semantics are identical.
_MAX_WAITS_PER_INST = 1


def _split_sync_waits(nc, cap=_MAX_WAITS_PER_INST):
    n_split = 0
    for fn in nc.m.functions:
        for bb in fn.blocks:
            insts = list(bb.instructions)
            out = []
            changed = False
            for ins in insts:
                si = ins.sync_info
                waits = list(si.on_wait) if (si is not None and si.on_wait) else []
                if len(waits) > cap:
                    head, tail = waits[: len(waits) - cap], waits[len(waits) - cap :]
                    for j, w in enumerate(head):
                        ev = mybir.InstEventSemaphore(
                            name=f"{ins.name}-sw{j}",
                            engine=ins.engine,
                            ins=[],
                            outs=[],
                            sync_info=bass_rust.SyncInfo(on_wait=[w], on_update=[]),
                        )
                        out.append(ev)
                        n_split += 1
                    si.on_wait = tail
                    changed = True
                out.append(ins)
            if changed:
                bb.instructions[:] = out
    return n_split


def _dram_row_bcast(handle, p, n):
    """AP reading DRAM vector [n] broadcast across p partitions."""
    return bass.AP(tensor=handle, offset=0, ap=[[0, p], [1, n]])


def _build(s, h, nh, sh, flags, split=True):
    """Build the per-core Bass program. flags: which bias/affine inputs matter."""
    hd = h // nh
    assert hd == 64, "head packing assumes head_dim 64 (2 heads per 128 partitions)"
    kt_n = h // 128  # contraction tiles over hidden dim
    kp_n = kt_n // 2  # DoubleRow pairs over hidden dim
    tt_n = s // 128  # key-token tiles
    tp_n = tt_n // 2  # key-token pair tiles
    qt_n = sh // 128  # query-token tiles
    iscale = 1.0 / WSCALE
    scale = 1.0 / float(np.sqrt(hd))

    nc = bass.Bass(target_bir_lowering=False)
    xt8d = nc.dram_tensor("xt8", [128, kp_n * 2 * s], F8, kind="ExternalInput")
    xbd = nc.dram_tensor("xb", [sh, h], BF, kind="ExternalInput")
    mask = nc.dram_tensor("mask", [s], FP, kind="ExternalInput")
    w8d = {
        n: nc.dram_tensor(n, [128, kp_n * 2 * h], F8, kind="ExternalInput")
        for n in ("wq8", "wk8", "wv8")
    }
    wod = nc.dram_tensor("wo8", [128, kp_n * 2 * h], F8, kind="ExternalInput")
    vec_dram = {
        n: nc.dram_tensor(n, [h], FP, kind="ExternalInput")
        for n in ("bq", "bk", "bv", "bo", "ln_gamma", "ln_beta")
        if flags[n]
    }
    out = nc.dram_tensor("out", [sh, h], FP, kind="ExternalOutput")
    dbg = {}
    if DEBUG_DUMP:
        dbg = {
            "kt_d": nc.dram_tensor("kt_d", [128, kt_n * s], BF, kind="ExternalOutput"),
            "qt_d": nc.dram_tensor("qt_d", [128, kt_n * sh], BF, kind="ExternalOutput"),
            "vsb_d": nc.dram_tensor("vsb_d", [128, tp_n * 2 * nh * 65], F8, kind="ExternalOutput"),
            "ctx_d": nc.dram_tensor("ctx_d", [128, (nh // 2) * sh], F8, kind="ExternalOutput"),
        }

    with TileContext(nc) as tc, ExitStack() as st_all:
        persist = st_all.enter_context(tc.tile_pool(name="persist", bufs=1))
        dram = st_all.enter_context(tc.tile_pool(name="dram", bufs=1, space="DRAM"))
        st_mid = st_all.enter_context(ExitStack())
        # attention-phase SBUF pools allocated low in the stack
        psb = st_mid.enter_context(tc.tile_pool(name="psb", bufs=2))
        cupool = st_mid.enter_context(tc.tile_pool(name="cupool", bufs=4))
        rpool = st_mid.enter_context(tc.tile_pool(name="rpool", bufs=2))
        rbpool = st_mid.enter_context(tc.tile_pool(name="rbpool", bufs=2))

        xt8 = persist.tile([128, kp_n, 2, s], F8)
        kt = persist.tile([128, kt_n, s], BF)
        qt = persist.tile([128, kt_n, sh], BF)
        vsb = persist.tile([128, tp_n, 2, nh * 65], F8)
        ctx_t = persist.tile([128, nh // 2, sh], F8)  # holds 64*ctx (fp8 range)
        # per-head sum(exp) rows for batched recip: partitions 0-1 = head%2,
        # free-dim slot = head//2 (engine ops need 32-aligned partition bases)
        dall = persist.tile([2, nh // 2, sh], FP)
        w8 = {n: persist.tile([128, kp_n, 2, h], F8, name=n) for n in ("wq8", "wk8", "wv8")}
        wo8 = persist.tile([128, kp_n, 2, h], F8)
        mask_sb = persist.tile([128, tt_n], FP)
        eps_sb = persist.tile([128, 1], FP)

        nc.vector.memset(eps_sb, EPS)
        nc.sync.dma_start(out=mask_sb, in_=mask[:].rearrange("(t p) -> p t", p=128))
        # DMA priority order: K/Q weights + X first (head 0 needs them), V, Wo.
        for kp in range(kp_n):  # chunked: spread across DMA queues
            nc.sync.dma_start(
                out=xt8[:, kp],
                in_=xt8d[:, kp * 2 * s : (kp + 1) * 2 * s].rearrange("p (b c) -> p b c", b=2),
            )
            nc.sync.dma_start(
                out=w8["wk8"][:, kp],
                in_=w8d["wk8"][:, kp * 2 * h : (kp + 1) * 2 * h].rearrange("p (b c) -> p b c", b=2),
            )
        nc.sync.dma_start(out=w8["wq8"], in_=w8d["wq8"][:, :].rearrange("p (a b c) -> p a b c", a=kp_n, b=2))
        nc.sync.dma_start(out=w8["wv8"], in_=w8d["wv8"][:, :].rearrange("p (a b c) -> p a b c", a=kp_n, b=2))
        nc.sync.dma_start(out=wo8, in_=wod[:, :].rearrange("p (a b c) -> p a b c", a=kp_n, b=2))
        xres_all = persist.tile([128, qt_n, h], BF, name="xres_all")
        for mq in range(qt_n):
            nc.sync.dma_start(
                out=xres_all[:, mq], in_=xbd[mq * 128 : (mq + 1) * 128, :]
            )

        # bias columns for Qt/Kt evictions (partition = output feature in tile)
        bias_cols = {}
        for name in ("bq", "bk"):
            if flags[name]:
                col = persist.tile([128, kt_n], FP, name=f"{name}_col")
                nc.sync.dma_start(
                    out=col, in_=vec_dram[name][:].rearrange("(t p) -> p t", p=128)
                )
                bias_cols[name] = col
        # rows broadcast across partitions for V/out bias and LN affine
        bcast = {}
        for name in ("bv", "bo", "ln_gamma", "ln_beta"):
            if flags[name]:
                t = persist.tile([128, h], FP, name=f"{name}_bc")
                nc.sync.dma_start(out=t, in_=_dram_row_bcast(vec_dram[name], 128, h))
                bcast[name] = t

        # ones columns in V (slot 64 of each 65-wide head block)
        for tp in range(tp_n):
            for i in range(2):
                v_view = vsb[:, tp, i, :].rearrange("p (a e) -> p a e", e=65)
                nc.vector.memset(v_view[:, :, 64:65], 1.0)

        with (
            tc.tile_pool(name="stps", bufs=2, space="PSUM") as stps,
            tc.tile_pool(name="pvps", bufs=1, space="PSUM") as pvps,
            tc.tile_pool(name="fillps", bufs=2, space="PSUM") as fillps,
        ):

            def kq_group(wname, dst, bias_col, m, n0):
                """K/Q projection group: 4 DoubleRow matmuls + scaled evict."""
                ps = fillps.tile([128, 512], FP, name="projp", tag="projp")
                for kp in range(kp_n):
                    nc.tensor.matmul(
                        ps,
                        w8[wname][:, kp, :, m * 128 : (m + 1) * 128],
                        xt8[:, kp, :, n0 : n0 + 512],
                        start=(kp == 0),
                        stop=(kp == kp_n - 1),
                        perf_mode=DR,
                    )
                if bias_col is not None:
                    nc.vector.tensor_scalar(
                        out=dst[:, m, n0 : n0 + 512],
                        in0=ps,
                        scalar1=iscale,
                        scalar2=bias_col[:, m : m + 1],
                        op0=OP.mult,
                        op1=OP.add,
                    )
                else:
                    nc.vector.tensor_scalar_mul(
                        out=dst[:, m, n0 : n0 + 512], in0=ps, scalar1=iscale
                    )

            def v_group(m, n0):
                """V projection group for token tile m, v-columns [n0, n0+512)."""
                ps = fillps.tile([128, 512], FP, name="projp", tag="projp")
                for kp in range(kp_n):
                    nc.tensor.matmul(
                        ps,
                        xt8[:, kp, :, m * 128 : (m + 1) * 128],
                        w8["wv8"][:, kp, :, n0 : n0 + 512],
                        start=(kp == 0),
                        stop=(kp == kp_n - 1),
                        perf_mode=DR,
                    )
                dst = vsb[:, m // 2, m % 2, :].rearrange("p (a e) -> p a e", e=65)[
                    :, n0 // 64 : n0 // 64 + 8, 0:64
                ]
                src = ps.rearrange("p (a e) -> p a e", e=64)
                if "bv" in bcast:
                    nc.vector.scalar_tensor_tensor(
                        out=dst,
                        in0=src,
                        scalar=iscale,
                        in1=bcast["bv"][:, n0 : n0 + 512].rearrange(
                            "p (a e) -> p a e", e=64
                        ),
                        op0=OP.mult,
                        op1=OP.add,
                    )
                else:
                    nc.vector.tensor_scalar_mul(out=dst, in0=src, scalar1=iscale)

            done = set()

            def run_task(t):
                if t in done:
                    return
                done.add(t)
                kind = t[0]
                if kind == "k":
                    kq_group("wk8", kt, bias_cols.get("bk"), t[1], t[2])
                elif kind == "q":
                    kq_group("wq8", qt, bias_cols.get("bq"), t[1], t[2])
                else:
                    v_group(t[1], t[2])

            # upfront: K/Q feature tile 0 (head 0+1 scores) only.
            for n0 in range(0, s, 512):
                run_task(("k", 0, n0))
            for n0 in range(0, sh, 512):
                run_task(("q", 0, n0))

            # fill queue: everything else, ordered so prerequisites stay ahead
            # of the heads that need them (forced emission is the safety net).
            fills = []
            for m in range(tt_n):  # V column block 0 (heads 0-7)
                fills.append(("v", m, 0))
            for m in range(1, kt_n):
                for n0 in range(0, s, 512):
                    fills.append(("k", m, n0))
                for n0 in range(0, sh, 512):
                    fills.append(("q", m, n0))
                if m <= 4 and nh > 8:  # V column block 1 (heads 8-15) early
                    for mm in range((m - 1) * 4, min(tt_n, m * 4)):
                        fills.append(("v", mm, 512))
            fills = [t for t in fills if t not in done]

            def pace_fill(k=1):
                n = 0
                while fills and n < k:
                    t = fills.pop(0)
                    if t not in done:
                        run_task(t)
                        n += 1

            # ---- attention ----
            # Deferred softmax normalization: each head frees its PV PSUM bank
            # with one [65,sh] copy and stashes the sum(exp) row; after every
            # 4th head ONE batched reciprocal (4 partitions in parallel) + a
            # DRAM-roundtrip broadcast + 4 ctx multiplies run, interleaved into
            # the NEXT head's pair loop so the DVE queue never blocks PV.
            from collections import deque

            pend = {}  # hh -> cu tile
            norm_tasks = deque()

            def group_norm(g):
                """Queue normalize work for heads 2g, 2g+1 (denoms in dall)."""
                def t_recip():
                    r = rpool.tile([2, sh], FP, name="recip")
                    nc.vector.reciprocal(r, dall[0:2, g, :])
                    rd = dram.tile([2, sh], FP, name="rdram", tag="rdram", bufs=2)
                    nc.sync.dma_start(out=rd, in_=r)
                    pend[("rd", g)] = rd
                norm_tasks.append(t_recip)

                def t_mult(hh):
                    rd = pend[("rd", g)]
                    rb = rbpool.tile([64, sh], FP, name="recipbc")
                    nc.sync.dma_start(
                        out=rb,
                        in_=bass.AP(
                            tensor=rd.tensor,
                            offset=rd.offset + (hh - 2 * g) * sh,
                            ap=[[0, 64], [1, sh]],
                        ),
                    )
                    mt, po = hh // 2, 64 * (hh % 2)
                    nc.vector.scalar_tensor_tensor(
                        out=ctx_t[po : po + 64, mt, :],
                        in0=pend.pop(hh)[0:64, :],
                        scalar=float(CTX_SCALE),
                        in1=rb,
                        op0=OP.mult,
                        op1=OP.mult,
                    )
                for hh in range(2 * g, 2 * g + 2):
                    norm_tasks.append(lambda hh=hh: t_mult(hh))

            for hh in range(nh):
                mt, po = hh // 2, 64 * (hh % 2)
                vn0 = (hh // 8) * 512
                hcol = hh * 65
                pv = pvps.tile([65, sh], FP, name="pvp")
                for tp in range(tp_n):
                    # forced prerequisites for this pair's PV
                    run_task(("v", 2 * tp, vn0))
                    run_task(("v", 2 * tp + 1, vn0))
                    if tp == 0:  # scores prerequisites for this head
                        for n0 in range(0, s, 512):
                            run_task(("k", mt, n0))
                        for n0 in range(0, sh, 512):
                            run_task(("q", mt, n0))
                    p2 = psb.tile([128, 2, sh], F8, name="pexp")
                    for i in range(2):
                        m = 2 * tp + i
                        stt = stps.tile([128, sh], FP, name="stp")
                        for n0 in range(0, sh, 512):
                            nc.tensor.matmul(
                                stt[:, n0 : n0 + 512],
                                kt[po : po + 64, mt, m * 128 : (m + 1) * 128],
                                qt[po : po + 64, mt, n0 : n0 + 512],
                                start=True,
                                stop=True,
                            )
                        nc.scalar.activation(
                            p2[:, i, :], stt, AF.Exp,
                            bias=mask_sb[:, m : m + 1], scale=scale,
                        )
                    for n0 in range(0, sh, 512):
                        nc.tensor.matmul(
                            pv[:, n0 : n0 + 512],
                            vsb[:, tp, :, hcol : hcol + 65],
                            p2[:, :, n0 : n0 + 512],
                            start=(tp == 0),
                            stop=(tp == tp_n - 1),
                            perf_mode=DR,
                        )
                    if hh > 0:
                        pace_fill(2 if hh < 6 else 1)
                    if norm_tasks:
                        norm_tasks.popleft()()
                # single f32 copy releases the PV PSUM bank; normalization is
                # deferred (runs while later heads stream).
                cu = cupool.tile([65, sh], FP, name="ctxu")
                nc.vector.tensor_copy(out=cu, in_=pv)
                pend[hh] = cu
                if hh < nh - 2:
                    nc.sync.dma_start(
                        out=dall[hh % 2 : hh % 2 + 1, hh // 2, :], in_=cu[64:65, :]
                    )
                    if hh % 2 == 1:
                        group_norm(hh // 2)
                else:
                    # last two heads: immediate chain, shortest path to O-proj
                    r = rpool.tile([2, sh], FP, name="recip")
                    nc.vector.reciprocal(r[0:1, :], cu[64:65, :])
                    rd = dram.tile([2, sh], FP, name="rdram", tag="rdram", bufs=2)
                    nc.sync.dma_start(out=rd[0:1, :], in_=r[0:1, :])
                    rb = rbpool.tile([64, sh], FP, name="recipbc")
                    nc.sync.dma_start(
                        out=rb,
                        in_=bass.AP(
                            tensor=rd.tensor, offset=rd.offset, ap=[[0, 64], [1, sh]]
                        ),
                    )
                    mt, po = hh // 2, 64 * (hh % 2)
                    nc.vector.scalar_tensor_tensor(
                        out=ctx_t[po : po + 64, mt, :],
                        in0=pend.pop(hh)[0:64, :],
                        scalar=float(CTX_SCALE),
                        in1=rb,
                        op0=OP.mult,
                        op1=OP.mult,
                    )
            while norm_tasks:
                norm_tasks.popleft()()
            while fills:
                pace_fill(len(fills))
            assert not pend or all(isinstance(k, tuple) for k in pend)

        if DEBUG_DUMP:
            nc.sync.dma_start(out=dbg["kt_d"][:, :], in_=kt.rearrange("p a c -> p (a c)"))
            nc.sync.dma_start(out=dbg["qt_d"][:, :], in_=qt.rearrange("p a c -> p (a c)"))
            nc.sync.dma_start(out=dbg["vsb_d"][:, :], in_=vsb.rearrange("p a b c -> p (a b c)"))
            nc.sync.dma_start(out=dbg["ctx_d"][:, :], in_=ctx_t.rearrange("p a c -> p (a c)"))

        st_mid.close()  # release attention pools before output phase

        # ---- output projection + residual + LayerNorm (natural layout) ----
        with (
            tc.tile_pool(name="ops", bufs=4, space="PSUM") as ops,
            tc.tile_pool(name="osb", bufs=2) as osb,
            tc.tile_pool(name="lnp", bufs=2) as lnp,
        ):
            for mg in range(0, qt_n, 2):
              pss2 = {mg: [], mg + 1: []}
              for m in (mg, mg + 1):
                for n0 in range(0, h, 512):
                    ps = ops.tile([128, 512], FP, name="op")
                    # ctx_t tile mt holds heads 2mt / 2mt+1 on partitions
                    # 0-63 / 64-127, exactly matching Wo rows mt*128..(mt+1)*128;
                    # DoubleRow pairs contract two mt tiles per pass.  The
                    # final pair (heads 12-15) is emitted last so these
                    # accumulations start before the tail normalize finishes.
                    for kp in range(kp_n - 1):
                        nc.tensor.matmul(
                            ps,
                            ctx_t[:, 2 * kp : 2 * kp + 2, m * 128 : (m + 1) * 128],
                            wo8[:, kp, :, n0 : n0 + 512],
                            start=(kp == 0),
                            stop=False,
                            perf_mode=DR,
                        )
                    pss2[m].append((n0, ps))
              for m in (mg, mg + 1):
                pss = pss2[m]
                for n0, ps in pss:
                    nc.tensor.matmul(
                        ps,
                        ctx_t[:, kt_n - 2 : kt_n, m * 128 : (m + 1) * 128],
                        wo8[:, kp_n - 1, :, n0 : n0 + 512],
                        start=False,
                        stop=True,
                        perf_mode=DR,
                    )
                xres = xres_all[:, m]
                o = osb.tile([128, h], FP, name="osum")
                for n0, ps in pss:
                    nc.vector.scalar_tensor_tensor(
                        out=o[:, n0 : n0 + 512],
                        in0=ps,
                        scalar=1.0 / (WSCALE * CTX_SCALE),
                        in1=xres[:, n0 : n0 + 512],
                        op0=OP.mult,
                        op1=OP.add,
                    )
                if "bo" in bcast:
                    nc.vector.tensor_add(out=o, in0=o, in1=bcast["bo"])
                nsub = (h + 511) // 512
                stats = lnp.tile([128, nsub, 6], FP, name="stats")
                for i in range(nsub):
                    nc.vector.bn_stats(
                        out=stats[:, i, :], in_=o[:, i * 512 : (i + 1) * 512]
                    )
                mv = lnp.tile([128, 2], FP, name="mv")
                nc.vector.bn_aggr(out=mv, in_=stats)
                std = lnp.tile([128, 1], FP, name="std")
                nc.scalar.activation(std, mv[:, 1:2], AF.Sqrt, bias=eps_sb)
                inv = lnp.tile([128, 1], FP, name="inv")
                nc.vector.reciprocal(inv, std)
                y = osb.tile([128, h], FP, name="yout")
                nc.vector.tensor_scalar(
                    out=y,
                    in0=o,
                    scalar1=mv[:, 0:1],
                    scalar2=inv,
                    op0=OP.subtract,
                    op1=OP.mult,
                )
                if "ln_gamma" in bcast:
                    nc.vector.tensor_mul(out=y, in0=y, in1=bcast["ln_gamma"])
                if "ln_beta" in bcast:
                    nc.vector.tensor_add(out=y, in0=y, in1=bcast["ln_beta"])
                nc.sync.dma_start(out=out[m * 128 : (m + 1) * 128, :], in_=y)

    if split:
        _split_sync_waits(nc)
    return nc


_NC_CACHE = {}


def _get_nc(s, h, nh, sh, flags):
    key = (s, h, nh, sh, tuple(sorted(flags.items())))
    if key not in _NC_CACHE:
        _NC_CACHE[key] = _build(s, h, nh, sh, flags)
    return _NC_CACHE[key]


def _pack_pairs(wt, h):
    """[h, n] f32 -> flat [128, (h/256)*2*n] fp8 in DoubleRow pair layout:
    out[p, kp, i, :] = wt[(2*kp + i)*128 + p, :]."""
    n = wt.shape[1]
    kp_n = h // 256
    a = wt.reshape(kp_n, 2, 128, n).transpose(2, 0, 1, 3).reshape(128, -1)
    return np.ascontiguousarray(a.astype(E4NP))


def _prepare(hidden_states, attention_mask, Wq, bq, Wk, bk, Wv, bv, Wo, bo, ln_gamma, ln_beta):
    hs = np.ascontiguousarray(np.asarray(hidden_states, dtype=np.float32))
    b_, s_, h_ = hs.shape
    nh_ = h_ // 64
    sh_ = s_ // 2
    am = np.asarray(attention_mask, dtype=np.float32).reshape(b_, s_)
    flags = {
        "bq": bool(np.any(np.asarray(bq))),
        "bk": bool(np.any(np.asarray(bk))),
        "bv": bool(np.any(np.asarray(bv))),
        "bo": bool(np.any(np.asarray(bo))),
        "ln_gamma": not bool(np.all(np.asarray(ln_gamma) == 1.0)),
        "ln_beta": bool(np.any(np.asarray(ln_beta))),
    }
    nc = _get_nc(s_, h_, nh_, sh_, flags)

    f32c = lambda a: np.ascontiguousarray(np.asarray(a, dtype=np.float32))
    shared = {
        "wq8": _pack_pairs(f32c(Wq) * WSCALE, h_),
        "wk8": _pack_pairs(f32c(Wk) * WSCALE, h_),
        "wv8": _pack_pairs(f32c(Wv) * WSCALE, h_),
        "wo8": _pack_pairs(f32c(Wo) * WSCALE, h_),
    }
    for name, arr in (
        ("bq", bq),
        ("bk", bk),
        ("bv", bv),
        ("bo", bo),
        ("ln_gamma", ln_gamma),
        ("ln_beta", ln_beta),
    ):
        if flags[name]:
            shared[name] = f32c(arr)

    in_maps = []
    for c in range(N_CORES):
        bb, half = c // 2, c % 2
        mine = slice(half * sh_, (half + 1) * sh_)
        other = slice((1 - half) * sh_, (2 - half) * sh_)
        xp = np.concatenate([hs[bb, mine], hs[bb, other]], axis=0)  # [s, h] f32
        mp = np.ascontiguousarray(np.concatenate([am[bb, mine], am[bb, other]]))
        in_maps.append(
            {
                "xt8": _pack_pairs(np.ascontiguousarray(xp.T), h_),
                "xb": np.ascontiguousarray(xp[:sh_].astype(BFNP)),
                "mask": mp,
                **shared,
            }
        )
    return nc, in_maps, (b_, s_, h_, sh_)


def _assemble(results, shape):
    b_, s_, h_, sh_ = shape
    out = np.empty((b_, s_, h_), dtype=np.float32)
    for c in range(N_CORES):
        bb, half = c // 2, c % 2
        out[bb, half * sh_ : (half + 1) * sh_] = results[c]["out"]
    return out


def kernel(**inputs) -> np.ndarray:
    nc, in_maps, shape = _prepare(**inputs)
    res = run_bass_kernel_spmd(nc, in_maps, core_ids=list(range(N_CORES)))
    return _assemble(res.results, shape)
